# revision 1
# baseline (speedup 1.0000x reference)
"""Trainium2 Bass kernel for the NT-Xent style contrastive loss.

loss = sum_j log(den_sum[j]) - (S1 . S2) / (N*T)
  den_sum[j] = sum_k (~mask[j,k]) * exp(sim(zn_j, zn_k) / T)
  S1 = sum_i z_i,  S2 = sum_j z_p_j   (z / zn / z_p row-L2-normalized)

Sharding: core c owns rows [c*1024, (c+1)*1024). Each core computes the
masked-exp row sums of its row-block of the 8192x8192 cosine-sim matrix
against all columns, plus partial sums for S1/S2. Host combines in f64.

Device pipeline per core (eye-mask fast path), organized as a pipeline
over 8 column groups of 1024 rows:
  - squares on GPSIMD, per-group row-norm reduces on DVE
  - inv_r via DVE Newton rsqrt (reciprocal-seeded, 2 iterations) so the
    ScalarE activation table never leaves the Exp set
  - zn (row-major bf16) = nodes * inv_r, then znT groups via PE tile
    transposes (bf16) interleaved with the main matmuls
  - sim row-block: bf16 matmuls (i-side left raw; its 1/r folded into
    the exp scale), PSUM f32, double-buffered [128, 1536] chunks
  - ScalarE activation(Exp, scale=inv_r_i/T, accum_out) fused row sums
  - mask handling: the expected input is eye(N) -> host subtracts
    exp(sim_jj/T) ~= e^2 per row. General fallback (any mask): separate
    build; DVE tensor_tensor_reduce of the exp rows against the bf16
    mask, subtracted per row on host.
"""

import os
import sys
import types
from contextlib import ExitStack

import numpy as np

sys.path.insert(0, "/opt/trn_rl_repo")

import ml_dtypes  # noqa: E402

import concourse.bass as bass  # noqa: E402
import concourse.tile as tile  # noqa: E402
from concourse import bacc, mybir  # noqa: E402
from concourse.bass_utils import run_bass_kernel_spmd  # noqa: E402
from concourse.masks import make_identity  # noqa: E402

N = 8192
D = 128
NCORES = 8
T = 0.5
R = N // NCORES        # rows per core
NB = R // 128          # i-blocks per core
NG = N // 1024         # column groups of 1024
F32 = mybir.dt.float32
BF16 = mybir.dt.bfloat16
AX = mybir.AxisListType
ALU = mybir.AluOpType
ACTF = mybir.ActivationFunctionType

# rsqrt seed: 1/sqrt(x) ~= A/x + B, minimax on x in [30, 400]
RSQ_A = 4.715
RSQ_B = 0.043133

# eye path k-chunking: 5 x 1536 + 512
CHUNKS = [(i * 1536, 1536) for i in range(5)] + [(7680, 512)]
NCH = len(CHUNKS)
# groups that must be transposed before chunk ci's matmuls run
PRE_TR = {0: [0, 1], 1: [2], 2: [3, 4], 3: [5], 4: [6, 7], 5: []}

LAST_EXEC_TIME_NS = None


def _install_trace_hook():
    """Make run_bass_kernel_spmd(trace=True) work under axon by supplying
    the antenv.axon_hooks module this image lacks."""
    try:
        if "antenv.axon_hooks" in sys.modules:
            return
        import antenv
        from trn_agent_boot.trn_boot import _ntff_profile_via_ctypes

        hook = _ntff_profile_via_ctypes("/opt/axon/libaxon_pjrt.so")
        m = types.ModuleType("antenv.axon_hooks")
        box = [hook]
        m.set_axon_ntff_profile_hook = lambda h: box.__setitem__(0, h)
        m.get_axon_ntff_profile_hook = lambda: box[0]
        sys.modules["antenv.axon_hooks"] = m
        antenv.axon_hooks = m
    except Exception:
        pass


def _bcast_inner(ap, n):
    """Broadcast a [P, F] AP to [P, F, n] with stride-0 innermost dim."""
    return bass.AP(tensor=ap.tensor, offset=ap.offset, ap=[*ap.ap, [0, n]])


def _newton_rsqrt(nc, pool, out, x, w):
    """out = 1/sqrt(x) elementwise, [128, w] f32, entirely on DVE.

    Seed A/x + B (~10% rel err on x in [30, 400]), then two Newton steps
    via scalar_tensor_tensor, whose (b - 1.5)*y form flips the sign each
    step; after an even number of steps the result is positive."""
    r = pool.tile([128, w], F32, tag="nt_r")
    nc.vector.reciprocal(r, x)
    y0 = pool.tile([128, w], F32, tag="nt_y0")
    nc.vector.tensor_scalar(
        out=y0, in0=r, scalar1=RSQ_A, scalar2=RSQ_B, op0=ALU.mult, op1=ALU.add
    )
    xh = pool.tile([128, w], F32, tag="nt_xh")
    nc.vector.tensor_scalar_mul(xh, x, 0.5)
    y = y0
    for it in range(2):
        a = pool.tile([128, w], F32, tag="nt_a")
        nc.vector.tensor_mul(a, y, y)
        b = pool.tile([128, w], F32, tag="nt_b")
        nc.vector.tensor_mul(b, a, xh)
        y2 = out if it == 1 else pool.tile([128, w], F32, tag="nt_y")
        nc.vector.scalar_tensor_tensor(
            out=y2, in0=b, scalar=1.5, in1=y, op0=ALU.subtract, op1=ALU.mult
        )
        y = y2
    return out


# eye-path chunking: 1024-wide lead chunk (needs only zn group 0), then
# 1536-wide, 1024 tail. PSUM: ps pool 2x[128,1536] (6 banks) + 1 shared
# bank for PE-transpose staging and the S1/S2 column-sum matmuls.
CHUNKS_EYE = [
    (0, 1024), (1024, 1536), (2560, 1536), (4096, 2048), (6144, 2048),
]
NCH_EYE = len(CHUNKS_EYE)
# transposes emitted ahead of need; all must be emitted before the PSUM
# pool swap that precedes the 2048-wide chunk columns
PRE_TR_EYE = {0: [0], 1: [1, 2], 2: [3, 4, 5, 6, 7], 3: [], 4: []}


def _build_eye():
    nc = bacc.Bacc(
        "TRN2", target_bir_lowering=False, debug=False, num_devices=NCORES
    )
    nodes_rm = nc.dram_tensor("nodes_rm", [N, D], F32, kind="ExternalInput").ap()
    own_rm = nc.dram_tensor("own_rm", [R, D], F32, kind="ExternalInput").ap()
    pair_rm = nc.dram_tensor("pair_rm", [R, D], F32, kind="ExternalInput").ap()
    den_out = nc.dram_tensor(
        "den", [128, NB * NCH_EYE], F32, kind="ExternalOutput"
    ).ap()
    s1_out = nc.dram_tensor("s1p", [1, R], F32, kind="ExternalOutput").ap()
    s2_out = nc.dram_tensor("s2p", [1, R], F32, kind="ExternalOutput").ap()

    # contiguous per-partition layout: row n = g*1024 + p*8 + t
    nodes_g = nodes_rm.rearrange("(g p t) d -> g p t d", p=128, t=NB)

    with tile.TileContext(nc) as tc, ExitStack() as ctx:
        pers = ctx.enter_context(tc.tile_pool(name="pers", bufs=1))
        grp = ctx.enter_context(tc.tile_pool(name="grp", bufs=1))
        junk = ctx.enter_context(tc.tile_pool(name="junk", bufs=2))
        small_psum = ExitStack()
        ps = small_psum.enter_context(tc.tile_pool(name="ps", bufs=2, space="PSUM"))
        ptx = small_psum.enter_context(
            tc.tile_pool(name="ptx", bufs=1, space="PSUM")
        )

        own_bf = pers.tile([128, R], BF16)
        inv_ri_T = pers.tile([128, NB], F32)
        den_sb = pers.tile([128, NCH_EYE, NB], F32)
        ident = pers.tile([128, 128], BF16)
        ones = pers.tile([128, 1], F32)
        s1sb = pers.tile([1, R], F32)
        s2sb = pers.tile([1, R], F32)

        # --- input DMAs on the sync ring: own/pair first, then groups
        op_rm = grp.tile([128, 2 * NB, D], F32)  # own rows | pair rows
        nc.sync.dma_start(
            out=op_rm[:, 0:NB, :],
            in_=own_rm.rearrange("(p t) d -> p t d", t=NB),
        )
        nc.sync.dma_start(
            out=op_rm[:, NB : 2 * NB, :],
            in_=pair_rm.rearrange("(p t) d -> p t d", t=NB),
        )
        rm_g = []
        for g in range(NG):
            t = grp.tile([128, NB, D], F32, tag=f"rm{g}", name=f"rm{g}")
            nc.sync.dma_start(out=t, in_=nodes_g[g])
            rm_g.append(t)

        make_identity(nc, ident)
        nc.vector.memset(ones, 1.0)

        # --- squares: own|pair, g0 on DVE, g1..g7 on gpsimd (parallel)
        sq_g = {}
        for g in range(1, NG):
            t = grp.tile([128, NB, D], F32, tag=f"sq{g}", name=f"sq{g}")
            nc.gpsimd.tensor_mul(t, rm_g[g], rm_g[g])
            sq_g[g] = t
        own_rm_bf = grp.tile([128, NB, D], BF16)
        nc.vector.tensor_copy(own_rm_bf, op_rm[:, 0:NB, :])
        sq_op = grp.tile([128, 2 * NB, D], F32)
        nc.vector.tensor_mul(sq_op, op_rm, op_rm)
        for g in range(1):
            t = grp.tile([128, NB, D], F32, tag=f"sq{g}", name=f"sq{g}")
            nc.vector.tensor_mul(t, rm_g[g], rm_g[g])
            sq_g[g] = t

        # --- all norms packed in one tile: [own|pair | g0..g7]
        # inv via one ScalarE Sqrt (its table loads while ScalarE is idle
        # anyway, before the Exp set) + one DVE reciprocal; the tiny-DVE-op
        # floor (~1.8us/op) makes Newton chains far more expensive here.
        n2_all = grp.tile([128, 2 * NB + NG * NB], F32)
        nc.vector.tensor_reduce(
            out=n2_all[:, 0 : 2 * NB], in_=sq_op, axis=AX.X, op=ALU.add
        )
        for g in range(NG):
            nc.vector.tensor_reduce(
                out=n2_all[:, (2 + g) * NB : (3 + g) * NB],
                in_=sq_g[g],
                axis=AX.X,
                op=ALU.add,
            )
        rnorm = grp.tile([128, 2 * NB + NG * NB], F32)
        nc.scalar.activation(out=rnorm, in_=n2_all, func=ACTF.Sqrt)
        inv_all = grp.tile([128, 2 * NB + NG * NB], F32)
        nc.vector.reciprocal(inv_all, rnorm)
        nc.vector.tensor_scalar_mul(inv_ri_T, inv_all[:, 0:NB], 1.0 / T)

        znT_g = [
            grp.tile([128, 1024], BF16, tag=f"znT{g}", name=f"znT{g}")
            for g in range(NG)
        ]
        zn_g = {}

        def group_zn(g):
            zn = grp.tile([128, NB, D], BF16, tag=f"zn{g}", name=f"zn{g}")
            nc.vector.tensor_mul(
                zn,
                rm_g[g],
                _bcast_inner(inv_all[:, (2 + g) * NB : (3 + g) * NB], D),
            )
            zn_g[g] = zn

        def emit_group_tr(src3d, dst, nmtag):
            pst = ptx.tile([128, NB, 128], BF16, tag="trx", name=f"pst{nmtag}")
            for t in range(NB):
                nc.tensor.transpose(pst[:, t, :], src3d[:, t, :], ident)
            nc.vector.tensor_copy(dst, pst)

        group_zn(0)
        emit_group_tr(own_rm_bf, own_bf, "own")
        emit_group_tr(zn_g[0], znT_g[0], "g0")
        for g in range(1, NG):
            group_zn(g)

        # S1/S2 scaled sums (DVE) -- consumed by PE matmuls later
        zsc = grp.tile([128, NB, D], F32)
        nc.vector.tensor_mul(
            zsc, op_rm[:, 0:NB, :], _bcast_inner(inv_all[:, 0:NB], D)
        )
        zpsc = grp.tile([128, NB, D], F32)
        nc.vector.tensor_mul(
            zpsc, op_rm[:, NB : 2 * NB, :],
            _bcast_inner(inv_all[:, NB : 2 * NB], D),
        )

        def emit_main_ci(ci, pool):
            off, w = CHUNKS_EYE[ci]
            for b in range(NB):
                p = pool.tile([128, w], F32, tag="ps", name=f"ps{ci}_{b}")
                for j in range(w // 512):
                    k0 = off + j * 512
                    g, goff = divmod(k0, 1024)
                    nc.tensor.matmul(
                        out=p[:, j * 512 : (j + 1) * 512],
                        lhsT=own_bf[:, b * 128 : (b + 1) * 128],
                        rhs=znT_g[g][:, goff : goff + 512],
                        start=True,
                        stop=True,
                    )
                jt = junk.tile([128, w], F32, tag="junk", name=f"jk{ci}_{b}")
                nc.scalar.activation(
                    out=jt,
                    in_=p,
                    func=ACTF.Exp,
                    scale=inv_ri_T[:, b : b + 1],
                    accum_out=den_sb[:, ci, b : b + 1],
                )

        done_tr = {0}
        for ci in range(3):
            for g in PRE_TR_EYE[ci]:
                if g not in done_tr:
                    emit_group_tr(zn_g[g], znT_g[g], f"g{g}")
                    done_tr.add(g)
            emit_main_ci(ci, ps)
            if ci == 1:
                # S1/S2 matmuls through the shared bank while PE has slack
                zsc_f = zsc.rearrange("p t d -> p (t d)")
                zpsc_f = zpsc.rearrange("p t d -> p (t d)")
                for src, dst, nm in ((zsc_f, s1sb, "s1"), (zpsc_f, s2sb, "s2")):
                    for h in range(R // 512):
                        sp = ptx.tile([1, 512], F32, tag="trx", name=f"{nm}p{h}")
                        nc.tensor.matmul(
                            out=sp,
                            lhsT=ones,
                            rhs=src[:, h * 512 : (h + 1) * 512],
                            start=True,
                            stop=True,
                        )
                        nc.vector.tensor_copy(
                            dst[:, h * 512 : (h + 1) * 512], sp
                        )

        # swap PSUM pools: free the 1536-pool + transpose bank, open 2x2048
        small_psum.close()
        psb = ctx.enter_context(tc.tile_pool(name="psb", bufs=2, space="PSUM"))
        emit_main_ci(3, psb)
        # ship all but the last chunk-column while ci4 still runs
        nc.gpsimd.dma_start(
            out=den_out[:, 0 : (NCH_EYE - 1) * NB],
            in_=den_sb[:, 0 : NCH_EYE - 1, :],
        )
        emit_main_ci(4, psb)

        nc.gpsimd.dma_start(out=den_out[:, (NCH_EYE - 1) * NB :], in_=den_sb[:, NCH_EYE - 1, :])
        nc.gpsimd.dma_start(out=s1_out, in_=s1sb)
        nc.gpsimd.dma_start(out=s2_out, in_=s2sb)

    nc.compile()
    return nc


def _build_general():
    """Correctness fallback for an arbitrary boolean mask (bf16 0/1 input).
    den correction per row: corr = sum_k mask[j,k] * E[j,k] via DVE
    tensor_tensor_reduce over the exp'd row block."""
    NCHG = 4
    CHG = N // NCHG
    nc = bacc.Bacc(
        "TRN2", target_bir_lowering=False, debug=False, num_devices=NCORES
    )
    nodes_rm = nc.dram_tensor("nodes_rm", [N, D], F32, kind="ExternalInput").ap()
    own_rm = nc.dram_tensor("own_rm", [R, D], F32, kind="ExternalInput").ap()
    pair_rm = nc.dram_tensor("pair_rm", [R, D], F32, kind="ExternalInput").ap()
    mask_bf = nc.dram_tensor("mask_bf", [R, N], BF16, kind="ExternalInput").ap()
    den_out = nc.dram_tensor("den", [128, NB * NCHG], F32, kind="ExternalOutput").ap()
    s1_out = nc.dram_tensor("s1p", [1, R], F32, kind="ExternalOutput").ap()
    s2_out = nc.dram_tensor("s2p", [1, R], F32, kind="ExternalOutput").ap()
    corr_out = nc.dram_tensor("corr", [128, NB], F32, kind="ExternalOutput").ap()

    NT = N // 128

    with tile.TileContext(nc) as tc, ExitStack() as ctx:
        persist = ctx.enter_context(tc.tile_pool(name="persist", bufs=1))
        znT = persist.tile([128, N], BF16)
        own_bf = persist.tile([128, R], BF16)
        inv_all = persist.tile([128, 80], F32)
        inv_ri_T = persist.tile([128, NB], F32)
        den_sb = persist.tile([128, NB, NCHG], F32)
        corr_sb = persist.tile([128, NB], F32)

        with (
            tc.tile_pool(name="pro", bufs=1) as pro,
            tc.tile_pool(name="psum_pro", bufs=1, space="PSUM") as psum_pro,
            tc.tile_pool(name="psum_tr", bufs=2, space="PSUM") as psum_tr,
        ):
            rm_sb = pro.tile([128, NT, D], F32)
            nc.sync.dma_start(
                out=rm_sb, in_=nodes_rm.rearrange("(t p) d -> p t d", p=128)
            )
            own_rm_sb = pro.tile([128, NB, D], F32)
            nc.sync.dma_start(
                out=own_rm_sb, in_=own_rm.rearrange("(t p) d -> p t d", p=128)
            )
            pair_rm_sb = pro.tile([128, NB, D], F32)
            nc.sync.dma_start(
                out=pair_rm_sb, in_=pair_rm.rearrange("(t p) d -> p t d", p=128)
            )

            ident = pro.tile([128, 128], BF16)
            make_identity(nc, ident)
            ones = pro.tile([128, 1], F32)
            nc.vector.memset(ones, 1.0)

            sq = pro.tile([128, NT, D], F32)
            nc.vector.tensor_mul(sq, rm_sb, rm_sb)
            norm2 = pro.tile([128, 80], F32)
            nc.vector.tensor_reduce(
                out=norm2[:, 0:NT], in_=sq, axis=AX.X, op=ALU.add
            )
            sq_own = pro.tile([128, NB, D], F32)
            nc.vector.tensor_mul(sq_own, own_rm_sb, own_rm_sb)
            nc.vector.tensor_reduce(
                out=norm2[:, NT : NT + NB], in_=sq_own, axis=AX.X, op=ALU.add
            )
            sq_pair = pro.tile([128, NB, D], F32)
            nc.vector.tensor_mul(sq_pair, pair_rm_sb, pair_rm_sb)
            nc.vector.tensor_reduce(
                out=norm2[:, NT + NB : NT + 2 * NB],
                in_=sq_pair,
                axis=AX.X,
                op=ALU.add,
            )
            norm2c = pro.tile([128, 80], F32)
            nc.vector.tensor_scalar_max(norm2c, norm2, 30.0)
            _newton_rsqrt(nc, pro, inv_all, norm2c, 80)
            inv_r_pt = inv_all[:, 0:NT]
            inv_ri = inv_all[:, NT : NT + NB]
            inv_rp = inv_all[:, NT + NB : NT + 2 * NB]

            nc.vector.tensor_scalar_mul(inv_ri_T, inv_ri, 1.0 / T)

            zn_rm = pro.tile([128, NT, D], BF16)
            nc.vector.tensor_mul(zn_rm, rm_sb, _bcast_inner(inv_r_pt, D))
            own_rm_bf = pro.tile([128, NB, D], BF16)
            nc.vector.tensor_copy(own_rm_bf, own_rm_sb)

            for g in range(NT // NB):
                pst = psum_tr.tile([128, NB, 128], BF16)
                for t in range(NB):
                    nc.tensor.transpose(
                        pst[:, t, :], zn_rm[:, g * NB + t, :], ident
                    )
                nc.vector.tensor_copy(
                    znT[:, g * NB * 128 : (g + 1) * NB * 128], pst
                )
            pst_o = psum_tr.tile([128, NB, 128], BF16)
            for t in range(NB):
                nc.tensor.transpose(pst_o[:, t, :], own_rm_bf[:, t, :], ident)
            nc.vector.tensor_copy(own_bf, pst_o)

            zsc = pro.tile([128, NB, D], F32)
            nc.vector.tensor_mul(zsc, own_rm_sb, _bcast_inner(inv_ri, D))
            zpsc = pro.tile([128, NB, D], F32)
            nc.vector.tensor_mul(zpsc, pair_rm_sb, _bcast_inner(inv_rp, D))
            s1ps = psum_pro.tile([1, R], F32)
            s2ps = psum_pro.tile([1, R], F32)
            zsc_f = zsc.rearrange("p t d -> p (t d)")
            zpsc_f = zpsc.rearrange("p t d -> p (t d)")
            for h in range(R // 512):
                nc.tensor.matmul(
                    out=s1ps[:, h * 512 : (h + 1) * 512],
                    lhsT=ones,
                    rhs=zsc_f[:, h * 512 : (h + 1) * 512],
                    start=True,
                    stop=True,
                )
                nc.tensor.matmul(
                    out=s2ps[:, h * 512 : (h + 1) * 512],
                    lhsT=ones,
                    rhs=zpsc_f[:, h * 512 : (h + 1) * 512],
                    start=True,
                    stop=True,
                )
            s1sb = pro.tile([1, R], F32)
            nc.vector.tensor_copy(s1sb, s1ps)
            s2sb = pro.tile([1, R], F32)
            nc.vector.tensor_copy(s2sb, s2ps)
            nc.sync.dma_start(out=s1_out, in_=s1sb)
            nc.sync.dma_start(out=s2_out, in_=s2sb)

        with (
            tc.tile_pool(name="psum_main", bufs=2, space="PSUM") as psum_main,
            tc.tile_pool(name="erow", bufs=2) as epool,
            tc.tile_pool(name="mrow", bufs=2) as mpool,
            tc.tile_pool(name="tjunk", bufs=2) as tjpool,
        ):
            for b in range(NB):
                erow = epool.tile([128, N], BF16)
                mrow = mpool.tile([128, N], BF16)
                nc.sync.dma_start(
                    out=mrow, in_=mask_bf[b * 128 : (b + 1) * 128, :]
                )
                for chi in range(NCHG):
                    p = psum_main.tile([128, CHG], F32)
                    for j in range(CHG // 512):
                        k0 = chi * CHG + j * 512
                        nc.tensor.matmul(
                            out=p[:, j * 512 : (j + 1) * 512],
                            lhsT=own_bf[:, b * 128 : (b + 1) * 128],
                            rhs=znT[:, k0 : k0 + 512],
                            start=True,
                            stop=True,
                        )
                    nc.scalar.activation(
                        out=erow[:, chi * CHG : (chi + 1) * CHG],
                        in_=p,
                        func=ACTF.Exp,
                        scale=inv_ri_T[:, b : b + 1],
                        accum_out=den_sb[:, b, chi : chi + 1],
                    )
                tj = tjpool.tile([128, N], BF16)
                nc.vector.tensor_tensor_reduce(
                    out=tj,
                    in0=erow,
                    in1=mrow,
                    scale=1.0,
                    scalar=0.0,
                    op0=ALU.mult,
                    op1=ALU.add,
                    accum_out=corr_sb[:, b : b + 1],
                )
            nc.sync.dma_start(out=den_out, in_=den_sb)
            nc.sync.dma_start(out=corr_out, in_=corr_sb)

    nc.compile()
    return nc


_PROGRAMS = {}


def _program(general: bool):
    if general not in _PROGRAMS:
        _PROGRAMS[general] = _build_general() if general else _build_eye()
    return _PROGRAMS[general]


def kernel(nodes, pair_nodes, nodes_labels, mask):
    global LAST_EXEC_TIME_NS
    nodes = np.ascontiguousarray(np.asarray(nodes), dtype=np.float32)
    pair = np.ascontiguousarray(np.asarray(pair_nodes), dtype=np.float32)
    mask = np.asarray(mask)
    assert nodes.shape == (N, D) and pair.shape == (N, D)

    mask_b = mask.astype(bool, copy=False)
    is_eye = bool(np.count_nonzero(mask_b) == N) and bool(
        mask_b.diagonal().all()
    )

    general = not is_eye
    if general:
        try:
            mask_bf = mask_b.astype(ml_dtypes.bfloat16)
            return _run(True, nodes, pair, mask_bf)
        except Exception:
            return _host_fallback(nodes, pair, mask_b)
    return _run(False, nodes, pair, None)


def _host_fallback(nodes, pair, mask_b):
    """Numpy reference for masks the device fallback cannot handle."""
    def norm_rows(x, eps):
        n = np.linalg.norm(x, axis=1, keepdims=True)
        return x / np.maximum(n, eps)

    n64 = nodes.astype(np.float64)
    p64 = pair.astype(np.float64)
    z = norm_rows(n64, 1e-12)
    zp = norm_rows(p64, 1e-12)
    zn = norm_rows(n64, 1e-8)
    logden = np.empty(N, dtype=np.float64)
    for i in range(0, N, 1024):
        sim = zn[i : i + 1024] @ zn.T
        den = (~mask_b[i : i + 1024] * np.exp(sim / T)).sum(1)
        logden[i : i + 1024] = np.log(den)
    loss = logden.sum() - float(z.sum(0) @ zp.sum(0)) / (N * T)
    return np.float32(loss)


def _run(general, nodes, pair, mask_bf):
    global LAST_EXEC_TIME_NS
    nc = _program(general)

    in_maps = []
    for c in range(NCORES):
        sl = slice(c * R, (c + 1) * R)
        m = {
            "nodes_rm": nodes,
            "own_rm": np.ascontiguousarray(nodes[sl]),
            "pair_rm": np.ascontiguousarray(pair[sl]),
        }
        if general:
            m["mask_bf"] = np.ascontiguousarray(mask_bf[sl])
        in_maps.append(m)

    trace = bool(os.environ.get("BASS_TRACE"))
    if trace:
        _install_trace_hook()
    res = run_bass_kernel_spmd(nc, in_maps, list(range(NCORES)), trace=trace)
    LAST_EXEC_TIME_NS = res.exec_time_ns

    nch = 4 if general else NCH_EYE
    den_rows = np.empty(N, dtype=np.float64)
    S1 = np.zeros(D, dtype=np.float64)
    S2 = np.zeros(D, dtype=np.float64)
    for c in range(NCORES):
        r = res.results[c]
        if general:
            den_pb = r["den"].astype(np.float64).reshape(128, NB, nch).sum(-1)
        else:
            den_pb = r["den"].astype(np.float64).reshape(128, nch, NB).sum(1)
        if general:
            den_pb -= r["corr"].astype(np.float64)
        else:
            den_pb -= np.exp(1.0 / T)
        if general:
            # row j = c*1024 + b*128 + p  ->  den_pb[p, b]
            den_rows[c * R : (c + 1) * R] = den_pb.T.reshape(R)
        else:
            # row j = c*1024 + p*8 + b  ->  den_pb[p, b]
            den_rows[c * R : (c + 1) * R] = den_pb.reshape(R)
        S1 += r["s1p"].astype(np.float64).reshape(NB, D).sum(0)
        S2 += r["s2p"].astype(np.float64).reshape(NB, D).sum(0)

    loss = np.log(den_rows).sum() - float(S1 @ S2) / (N * T)
    return np.float32(loss)



# revision 9
# speedup vs baseline: 1.2859x; 1.2859x over previous
"""Trainium2 Bass kernel for the NT-Xent style contrastive loss.

loss = sum_j log(den_sum[j]) - (S1 . S2) / (N*T)
  den_sum[j] = sum_k (~mask[j,k]) * exp(sim(zn_j, zn_k) / T)
  S1 = sum_i z_i,  S2 = sum_j z_p_j   (z / zn / z_p row-L2-normalized)

Eye-mask fast path exploits the SYMMETRY of E = exp(zn zn^T / T): only the
upper-triangle 1024x1024 blocks of the 8192x8192 matrix are exponentiated.
Row sums come free from the ScalarE activation accum_out; column sums of
each block (the mirrored rows' contributions) are ones-matmuls on the PE.

Work is balanced 36 strip-activations per core (33280 exp-columns, vs
65536 for the non-symmetric version):
  - diag block (c,c): triangle strips t: cols [t*128, 1024), rowsum via
    accum; colsum over cols [(t+1)*128, 1024) (excludes own 128-subtile
    whose mirrors are computed directly).
  - cyclic blocks (c, c+k) k=1..3: 8 full strips each, rowsum+colsum.
  - distance-4 pair {c, c+-4}: split by row halves between its two
    endpoint cores via dedicated gP (lhsT rows) / gQ (rhs cols) input
    slots; the host feeds core c>=4 the OTHER half's t-slices so the
    SPMD program stays uniform.
Both matmul operands are bf16 zn (rows pre-normalized), so the exp scale
is the constant 1/T*... = 2.0 and any core can run any strip. The host
combines row/col partials in f64 and subtracts the exact self-term.
"""

import os
import sys
import types
from contextlib import ExitStack

import numpy as np

sys.path.insert(0, "/opt/trn_rl_repo")

import ml_dtypes  # noqa: E402

import concourse.bass as bass  # noqa: E402
import concourse.tile as tile  # noqa: E402
from concourse import bacc, mybir  # noqa: E402
from concourse.bass_utils import run_bass_kernel_spmd  # noqa: E402
from concourse.masks import make_identity  # noqa: E402

N = 8192
D = 128
NCORES = 8
T = 0.5
R = N // NCORES        # rows per core
NB = R // 128          # i-blocks (strips) per 1024-row group
F32 = mybir.dt.float32
BF16 = mybir.dt.bfloat16
AX = mybir.AxisListType
ALU = mybir.AluOpType
ACTF = mybir.ActivationFunctionType

# rsqrt seed: 1/sqrt(x) ~= A/x + B, minimax on x in [30, 400]
RSQ_A = 4.715
RSQ_B = 0.043133

LAST_EXEC_TIME_NS = None


def _install_trace_hook():
    """Make run_bass_kernel_spmd(trace=True) work under axon by supplying
    the antenv.axon_hooks module this image lacks."""
    try:
        if "antenv.axon_hooks" in sys.modules:
            return
        import antenv
        from trn_agent_boot.trn_boot import _ntff_profile_via_ctypes

        hook = _ntff_profile_via_ctypes("/opt/axon/libaxon_pjrt.so")
        m = types.ModuleType("antenv.axon_hooks")
        box = [hook]
        m.set_axon_ntff_profile_hook = lambda h: box.__setitem__(0, h)
        m.get_axon_ntff_profile_hook = lambda: box[0]
        sys.modules["antenv.axon_hooks"] = m
        antenv.axon_hooks = m
    except Exception:
        pass


def _bcast_inner(ap, n):
    """Broadcast a [P, F] AP to [P, F, n] with stride-0 innermost dim."""
    return bass.AP(tensor=ap.tensor, offset=ap.offset, ap=[*ap.ap, [0, n]])


def _newton_rsqrt(nc, pool, out, x, w):
    """out = 1/sqrt(x) elementwise, [128, w] f32, entirely on DVE.

    Seed A/x + B (~10% rel err on x in [30, 400]), then two Newton steps
    via scalar_tensor_tensor, whose (b - 1.5)*y form flips the sign each
    step; after an even number of steps the result is positive."""
    r = pool.tile([128, w], F32, tag="nt_r", name="nt_r")
    nc.vector.reciprocal(r, x)
    y0 = pool.tile([128, w], F32, tag="nt_y0", name="nt_y0")
    nc.vector.tensor_scalar(
        out=y0, in0=r, scalar1=RSQ_A, scalar2=RSQ_B, op0=ALU.mult, op1=ALU.add
    )
    xh = pool.tile([128, w], F32, tag="nt_xh", name="nt_xh")
    nc.vector.tensor_scalar_mul(xh, x, 0.5)
    y = y0
    for it in range(2):
        a = pool.tile([128, w], F32, tag="nt_a", name="nt_a")
        nc.vector.tensor_mul(a, y, y)
        b = pool.tile([128, w], F32, tag="nt_b", name="nt_b")
        nc.vector.tensor_mul(b, a, xh)
        y2 = out if it == 1 else pool.tile([128, w], F32, tag="nt_y", name="nt_y")
        nc.vector.scalar_tensor_tensor(
            out=y2, in0=b, scalar=1.5, in1=y, op0=ALU.subtract, op1=ALU.mult
        )
        y = y2
    return out


def _split512(lo, hi):
    """Split [lo, hi) at multiples of 512 (PSUM bank boundaries)."""
    out = []
    a = lo
    while a < hi:
        b = min((a // 512 + 1) * 512, hi)
        out.append((a, b))
        a = b
    return out


# column layout within the packed norm tiles [128, 52]:
# g0 [0:8) g1 [8:16) g2 [16:24) g3 [24:32) Q [32:40) P [40:44) pair [44:52)
_NCOL = {"g0": (0, 8), "g1": (8, 16), "g2": (16, 24), "g3": (24, 32),
         "gq": (32, 40), "gp": (40, 44), "pr": (44, 52)}

# colsum dram layout: diag 896 | k1 1024 | k2 1024 | k3 1024 | P 1024
_COL_OFF = {"diag": 0, "k1": 896, "k2": 1920, "k3": 2944, "p": 3968}


def _build_sym():
    nc = bacc.Bacc(
        "TRN2", target_bir_lowering=False, debug=False, num_devices=NCORES
    )
    g_in = [
        nc.dram_tensor(f"g{j}", [128, NB, D], F32, kind="ExternalInput").ap()
        for j in range(4)
    ]
    gq_in = nc.dram_tensor("gq", [128, NB, D], F32, kind="ExternalInput").ap()
    gp_in = nc.dram_tensor("gp", [128, 4, D], F32, kind="ExternalInput").ap()
    pr_in = nc.dram_tensor("pr", [128, NB, D], F32, kind="ExternalInput").ap()
    den_out = nc.dram_tensor("den", [128, 36], F32, kind="ExternalOutput").ap()
    col_out = nc.dram_tensor("col", [1, 4992], F32, kind="ExternalOutput").ap()
    s1_out = nc.dram_tensor("s1p", [1, R], F32, kind="ExternalOutput").ap()
    s2_out = nc.dram_tensor("s2p", [1, R], F32, kind="ExternalOutput").ap()

    with tile.TileContext(nc) as tc, ExitStack() as ctx:
        pers = ctx.enter_context(tc.tile_pool(name="pers", bufs=1))
        spool = ctx.enter_context(tc.tile_pool(name="spool", bufs=2))
        epool = ctx.enter_context(tc.tile_pool(name="epool", bufs=3))
        depool = ctx.enter_context(tc.tile_pool(name="depool", bufs=8))
        pmm = ctx.enter_context(tc.tile_pool(name="pmm", bufs=2, space="PSUM"))
        pcol = ctx.enter_context(tc.tile_pool(name="pcol", bufs=1, space="PSUM"))
        ptx = ctx.enter_context(tc.tile_pool(name="ptx", bufs=2, space="PSUM"))

        ident = pers.tile([128, 128], BF16)
        ones_bf = pers.tile([128, 1], BF16)
        den_sb = pers.tile([128, 36], F32)
        n2 = pers.tile([128, 52], F32)
        n2c = pers.tile([128, 52], F32)
        inv = pers.tile([128, 52], F32)
        col_sb = pers.tile([1, 4992], F32)
        s_sb = pers.tile([1, 2 * R], F32)

        rm = {}
        for nm, ap_in, nt in (
            ("g0", g_in[0], NB), ("g1", g_in[1], NB), ("g2", g_in[2], NB),
            ("g3", g_in[3], NB), ("gq", gq_in, NB), ("gp", gp_in, 4),
            ("pr", pr_in, NB),
        ):
            rm[nm] = pers.tile([128, nt, D], F32, name=f"rm_{nm}")
        zn = {nm: pers.tile([128, nt, D], BF16, name=f"zn_{nm}")
              for nm, nt in (("g0", NB), ("g1", NB), ("g2", NB), ("g3", NB),
                             ("gq", NB), ("gp", 4), ("pr", NB))}
        znT = {nm: pers.tile([128, nt * 128], BF16, name=f"znT_{nm}")
               for nm, nt in (("g0", NB), ("g1", NB), ("g2", NB), ("g3", NB),
                              ("gq", NB), ("gp", 4))}

        # ---- input DMAs: g0/g1 race in parallel on separate rings so the
        # first two blocks' norm chain can run fused
        nc.sync.dma_start(out=rm["g0"], in_=g_in[0])
        nc.sync.dma_start(out=rm["g2"], in_=g_in[2])
        nc.sync.dma_start(out=rm["g3"], in_=g_in[3])
        nc.scalar.dma_start(out=rm["g1"], in_=g_in[1])
        nc.scalar.dma_start(out=rm["gq"], in_=gq_in)
        nc.scalar.dma_start(out=rm["gp"], in_=gp_in)
        nc.scalar.dma_start(out=rm["pr"], in_=pr_in)

        make_identity(nc, ident)
        nc.vector.memset(ones_bf, 1.0)
        # warm the Exp table while everything else is still loading
        junk1 = pers.tile([128, 1], F32)
        nc.scalar.activation(out=junk1, in_=ones_bf, func=ACTF.Exp)

        # ---- norms: squares + reduces on GPSIMD, rsqrt chains + zn on DVE
        def sq_red(nm):
            nt = rm[nm].shape[1]
            sq = spool.tile([128, nt, D], F32, tag="sq", name=f"sq_{nm}")
            nc.gpsimd.tensor_mul(sq, rm[nm], rm[nm])
            a, b = _NCOL[nm]
            nc.vector.tensor_reduce(
                out=n2[:, a:b], in_=sq, axis=AX.X, op=ALU.add
            )

        def chain(*nms):
            a = min(_NCOL[nm][0] for nm in nms)
            b = max(_NCOL[nm][1] for nm in nms)
            nc.vector.tensor_scalar_max(n2c[:, a:b], n2[:, a:b], 30.0)
            _newton_rsqrt(nc, pers, inv[:, a:b], n2c[:, a:b], b - a)
            for nm in nms:
                x, y = _NCOL[nm]
                nc.vector.tensor_mul(
                    zn[nm], rm[nm], _bcast_inner(inv[:, x:y], D)
                )

        def transposes(nm, order, copier, percol=True):
            nt = zn[nm].shape[1]
            pst = ptx.tile([128, nt, 128], BF16, tag="trx", name=f"pst_{nm}")
            for t in order:
                nc.tensor.transpose(pst[:, t, :], zn[nm][:, t, :], ident)
                if percol:
                    copier.tensor_copy(
                        znT[nm][:, t * 128:(t + 1) * 128], pst[:, t, :]
                    )
            if not percol:
                copier.tensor_copy(znT[nm], pst)

        sq_red("g0")
        sq_red("g1")
        chain("g0", "g1")
        # per-tile DVE copies so diag strips start as soon as the first
        # (narrowest) strip's operands exist
        transposes("g0", range(NB - 1, -1, -1), nc.vector)

        # ---- strip machinery
        def strip(lhsT_src, lt, rhs_src, lo, hi, slot, epool_, etag):
            w = hi - lo
            p = pmm.tile([128, w], F32, tag="ps", name=f"ps_{slot}")
            for a, b in _split512(0, w):
                nc.tensor.matmul(
                    out=p[:, a:b],
                    lhsT=lhsT_src[:, lt * 128:(lt + 1) * 128],
                    rhs=rhs_src[:, lo + a: lo + b],
                    start=True,
                    stop=True,
                )
            e = epool_.tile([128, w], BF16, tag=etag, name=f"e_{slot}")
            nc.scalar.activation(
                out=e, in_=p, func=ACTF.Exp, scale=1.0 / T,
                accum_out=den_sb[:, slot:slot + 1],
            )
            return e

        def colsum_mms(ctile, e, e_off, lo, hi, first_banks, last_banks):
            """ctile[lo:hi) += ones^T @ e[:, e_off + (.-lo)], bank-aware flags.
            first_banks/last_banks: sets of bank indices for which this is
            the first / last accumulating matmul."""
            for a, b in _split512(lo, hi):
                bank = a // 512
                nc.tensor.matmul(
                    out=ctile[:, a:b],
                    lhsT=ones_bf,
                    rhs=e[:, e_off + a - lo: e_off + b - lo],
                    start=bank in first_banks,
                    stop=bank in last_banks,
                )

        # ---- diag block: triangle strips, widest-last; E retained for the
        # end-of-kernel colsum pass (t=0 initializes the full accumulator)
        diag_e = {}
        for t in range(NB - 1, -1, -1):
            diag_e[t] = strip(
                znT["g0"], t, znT["g0"], t * 128, 1024, t, depool, "de"
            )

        sq_red("g2")
        sq_red("gq")
        sq_red("gp")
        transposes("g1", range(NB), nc.vector, percol=False)

        # ---- cyclic blocks k=1..3 + pair block: one-ahead mm emission so
        # PE never in-order-stalls ScalarE
        def block(lhsT_src, rhs_src, slot0, nstrips, ckey, extra=None):
            ctile = pcol.tile([1, 1024], F32, tag="col", name=f"c_{ckey}")
            es = {}
            es[0] = strip(lhsT_src, 0, rhs_src, 0, 1024, slot0, epool, "e")
            for s in range(1, nstrips + 1):
                if s <= nstrips - 1:
                    es[s] = strip(
                        lhsT_src, s, rhs_src, 0, 1024, slot0 + s, epool, "e"
                    )
                if extra is not None and s - 1 in extra:
                    extra[s - 1]()
                colsum_mms(
                    ctile, es[s - 1], 0, 0, 1024,
                    first_banks={0, 1} if s - 1 == 0 else set(),
                    last_banks={0, 1} if s - 1 == nstrips - 1 else set(),
                )
                del es[s - 1]
            nc.vector.tensor_copy(
                col_sb[:, _COL_OFF[ckey]:_COL_OFF[ckey] + 1024], ctile
            )

        block(znT["g0"], znT["g1"], 8, NB, "k1", extra={
            2: lambda: (sq_red("g3"), chain("g2"))[-1],
            4: lambda: transposes("g2", range(NB), nc.vector, percol=False),
            6: lambda: (sq_red("pr"), chain("gq", "gp"))[-1],
        })
        block(znT["g0"], znT["g2"], 16, NB, "k2", extra={
            2: lambda: chain("g3"),
            4: lambda: transposes("g3", range(NB), nc.vector, percol=False),
        })
        block(znT["g0"], znT["g3"], 24, NB, "k3", extra={
            2: lambda: transposes("gq", range(NB), nc.vector, percol=False),
            5: lambda: (chain("pr"),
                        transposes("gp", range(4), nc.vector,
                                   percol=False))[-1],
        })
        block(znT["gp"], znT["gq"], 32, 4, "p")

        # rowsums done -> ship them
        nc.gpsimd.dma_start(out=den_out, in_=den_sb)

        # ---- diag colsums: cols [128, 1024) of the group, ascending so
        # strip 0 (full width) opens both accumulation banks
        dcol = pcol.tile([1, 896], F32, tag="col", name="c_diag")
        for t in range(NB - 1):
            lo = t * 128
            first = {0, 1} if t == 0 else set()
            last = set()
            if t == 3:
                last.add(0)       # bank 0: writers t=0..3
            if t == NB - 2:
                last.add(1)       # bank 1: writers t=0..6
            colsum_mms(dcol, diag_e[t], 128, lo, 896, first, last)
        nc.vector.tensor_copy(col_sb[:, 0:896], dcol)
        nc.gpsimd.dma_start(out=col_out, in_=col_sb)

        # ---- S1/S2 partials: column sums of zn(own rows) / zn(pair rows)
        zn0_f = zn["g0"].rearrange("p t d -> p (t d)")
        znp_f = zn["pr"].rearrange("p t d -> p (t d)")
        for src, off in ((zn0_f, 0), (znp_f, R)):
            sp = pcol.tile([1, R], F32, tag="col", name=f"s_{off}")
            for a, b in _split512(0, R):
                nc.tensor.matmul(
                    out=sp[:, a:b], lhsT=ones_bf, rhs=src[:, a:b],
                    start=True, stop=True,
                )
            nc.vector.tensor_copy(s_sb[:, off:off + R], sp)
        nc.gpsimd.dma_start(out=s1_out, in_=s_sb[:, 0:R])
        nc.gpsimd.dma_start(out=s2_out, in_=s_sb[:, R:2 * R])

    nc.compile()
    return nc


def _build_general():
    """Correctness fallback for an arbitrary boolean mask (bf16 0/1 input).
    den correction per row: corr = sum_k mask[j,k] * E[j,k] via DVE
    tensor_tensor_reduce over the exp'd row block."""
    NCHG = 4
    CHG = N // NCHG
    nc = bacc.Bacc(
        "TRN2", target_bir_lowering=False, debug=False, num_devices=NCORES
    )
    nodes_rm = nc.dram_tensor("nodes_rm", [N, D], F32, kind="ExternalInput").ap()
    own_rm = nc.dram_tensor("own_rm", [R, D], F32, kind="ExternalInput").ap()
    pair_rm = nc.dram_tensor("pair_rm", [R, D], F32, kind="ExternalInput").ap()
    mask_bf = nc.dram_tensor("mask_bf", [R, N], BF16, kind="ExternalInput").ap()
    den_out = nc.dram_tensor("den", [128, NB * NCHG], F32, kind="ExternalOutput").ap()
    s1_out = nc.dram_tensor("s1p", [1, R], F32, kind="ExternalOutput").ap()
    s2_out = nc.dram_tensor("s2p", [1, R], F32, kind="ExternalOutput").ap()
    corr_out = nc.dram_tensor("corr", [128, NB], F32, kind="ExternalOutput").ap()

    NT = N // 128

    with tile.TileContext(nc) as tc, ExitStack() as ctx:
        persist = ctx.enter_context(tc.tile_pool(name="persist", bufs=1))
        znT = persist.tile([128, N], BF16)
        own_bf = persist.tile([128, R], BF16)
        inv_all = persist.tile([128, 80], F32)
        inv_ri_T = persist.tile([128, NB], F32)
        den_sb = persist.tile([128, NB, NCHG], F32)
        corr_sb = persist.tile([128, NB], F32)

        with (
            tc.tile_pool(name="pro", bufs=1) as pro,
            tc.tile_pool(name="psum_pro", bufs=1, space="PSUM") as psum_pro,
            tc.tile_pool(name="psum_tr", bufs=2, space="PSUM") as psum_tr,
        ):
            rm_sb = pro.tile([128, NT, D], F32)
            nc.sync.dma_start(
                out=rm_sb, in_=nodes_rm.rearrange("(t p) d -> p t d", p=128)
            )
            own_rm_sb = pro.tile([128, NB, D], F32)
            nc.sync.dma_start(
                out=own_rm_sb, in_=own_rm.rearrange("(t p) d -> p t d", p=128)
            )
            pair_rm_sb = pro.tile([128, NB, D], F32)
            nc.sync.dma_start(
                out=pair_rm_sb, in_=pair_rm.rearrange("(t p) d -> p t d", p=128)
            )

            ident = pro.tile([128, 128], BF16)
            make_identity(nc, ident)
            ones = pro.tile([128, 1], F32)
            nc.vector.memset(ones, 1.0)

            sq = pro.tile([128, NT, D], F32)
            nc.vector.tensor_mul(sq, rm_sb, rm_sb)
            norm2 = pro.tile([128, 80], F32)
            nc.vector.tensor_reduce(
                out=norm2[:, 0:NT], in_=sq, axis=AX.X, op=ALU.add
            )
            sq_own = pro.tile([128, NB, D], F32)
            nc.vector.tensor_mul(sq_own, own_rm_sb, own_rm_sb)
            nc.vector.tensor_reduce(
                out=norm2[:, NT: NT + NB], in_=sq_own, axis=AX.X, op=ALU.add
            )
            sq_pair = pro.tile([128, NB, D], F32)
            nc.vector.tensor_mul(sq_pair, pair_rm_sb, pair_rm_sb)
            nc.vector.tensor_reduce(
                out=norm2[:, NT + NB: NT + 2 * NB],
                in_=sq_pair,
                axis=AX.X,
                op=ALU.add,
            )
            norm2c = pro.tile([128, 80], F32)
            nc.vector.tensor_scalar_max(norm2c, norm2, 30.0)
            _newton_rsqrt(nc, pro, inv_all, norm2c, 80)
            inv_r_pt = inv_all[:, 0:NT]
            inv_ri = inv_all[:, NT: NT + NB]
            inv_rp = inv_all[:, NT + NB: NT + 2 * NB]

            nc.vector.tensor_scalar_mul(inv_ri_T, inv_ri, 1.0 / T)

            zn_rm = pro.tile([128, NT, D], BF16)
            nc.vector.tensor_mul(zn_rm, rm_sb, _bcast_inner(inv_r_pt, D))
            own_rm_bf = pro.tile([128, NB, D], BF16)
            nc.vector.tensor_copy(own_rm_bf, own_rm_sb)

            for g in range(NT // NB):
                pst = psum_tr.tile([128, NB, 128], BF16)
                for t in range(NB):
                    nc.tensor.transpose(
                        pst[:, t, :], zn_rm[:, g * NB + t, :], ident
                    )
                nc.vector.tensor_copy(
                    znT[:, g * NB * 128:(g + 1) * NB * 128], pst
                )
            pst_o = psum_tr.tile([128, NB, 128], BF16)
            for t in range(NB):
                nc.tensor.transpose(pst_o[:, t, :], own_rm_bf[:, t, :], ident)
            nc.vector.tensor_copy(own_bf, pst_o)

            zsc = pro.tile([128, NB, D], F32)
            nc.vector.tensor_mul(zsc, own_rm_sb, _bcast_inner(inv_ri, D))
            zpsc = pro.tile([128, NB, D], F32)
            nc.vector.tensor_mul(zpsc, pair_rm_sb, _bcast_inner(inv_rp, D))
            s1ps = psum_pro.tile([1, R], F32)
            s2ps = psum_pro.tile([1, R], F32)
            zsc_f = zsc.rearrange("p t d -> p (t d)")
            zpsc_f = zpsc.rearrange("p t d -> p (t d)")
            for h in range(R // 512):
                nc.tensor.matmul(
                    out=s1ps[:, h * 512:(h + 1) * 512],
                    lhsT=ones,
                    rhs=zsc_f[:, h * 512:(h + 1) * 512],
                    start=True,
                    stop=True,
                )
                nc.tensor.matmul(
                    out=s2ps[:, h * 512:(h + 1) * 512],
                    lhsT=ones,
                    rhs=zpsc_f[:, h * 512:(h + 1) * 512],
                    start=True,
                    stop=True,
                )
            s1sb = pro.tile([1, R], F32)
            nc.vector.tensor_copy(s1sb, s1ps)
            s2sb = pro.tile([1, R], F32)
            nc.vector.tensor_copy(s2sb, s2ps)
            nc.sync.dma_start(out=s1_out, in_=s1sb)
            nc.sync.dma_start(out=s2_out, in_=s2sb)

        with (
            tc.tile_pool(name="psum_main", bufs=2, space="PSUM") as psum_main,
            tc.tile_pool(name="erow", bufs=2) as epool,
            tc.tile_pool(name="mrow", bufs=2) as mpool,
            tc.tile_pool(name="tjunk", bufs=2) as tjpool,
        ):
            for b in range(NB):
                erow = epool.tile([128, N], BF16)
                mrow = mpool.tile([128, N], BF16)
                nc.sync.dma_start(
                    out=mrow, in_=mask_bf[b * 128:(b + 1) * 128, :]
                )
                for chi in range(NCHG):
                    p = psum_main.tile([128, CHG], F32)
                    for j in range(CHG // 512):
                        k0 = chi * CHG + j * 512
                        nc.tensor.matmul(
                            out=p[:, j * 512:(j + 1) * 512],
                            lhsT=own_bf[:, b * 128:(b + 1) * 128],
                            rhs=znT[:, k0: k0 + 512],
                            start=True,
                            stop=True,
                        )
                    nc.scalar.activation(
                        out=erow[:, chi * CHG:(chi + 1) * CHG],
                        in_=p,
                        func=ACTF.Exp,
                        scale=inv_ri_T[:, b: b + 1],
                        accum_out=den_sb[:, b, chi: chi + 1],
                    )
                tj = tjpool.tile([128, N], BF16)
                nc.vector.tensor_tensor_reduce(
                    out=tj,
                    in0=erow,
                    in1=mrow,
                    scale=1.0,
                    scalar=0.0,
                    op0=ALU.mult,
                    op1=ALU.add,
                    accum_out=corr_sb[:, b: b + 1],
                )
            nc.sync.dma_start(out=den_out, in_=den_sb)
            nc.sync.dma_start(out=corr_out, in_=corr_sb)

    nc.compile()
    return nc


_PROGRAMS = {}


def _program(general: bool):
    if general not in _PROGRAMS:
        _PROGRAMS[general] = _build_general() if general else _build_sym()
    return _PROGRAMS[general]


def kernel(nodes, pair_nodes, nodes_labels, mask):
    global LAST_EXEC_TIME_NS
    nodes = np.ascontiguousarray(np.asarray(nodes), dtype=np.float32)
    pair = np.ascontiguousarray(np.asarray(pair_nodes), dtype=np.float32)
    mask = np.asarray(mask)
    assert nodes.shape == (N, D) and pair.shape == (N, D)

    mask_b = mask.astype(bool, copy=False)
    is_eye = bool(np.count_nonzero(mask_b) == N) and bool(
        mask_b.diagonal().all()
    )

    if not is_eye:
        try:
            mask_bf = mask_b.astype(ml_dtypes.bfloat16)
            return _run_general(nodes, pair, mask_bf)
        except Exception:
            return _host_fallback(nodes, pair, mask_b)
    return _run_eye(nodes, pair)


def _host_fallback(nodes, pair, mask_b):
    """Numpy reference for masks the device fallback cannot handle."""
    def norm_rows(x, eps):
        n = np.linalg.norm(x, axis=1, keepdims=True)
        return x / np.maximum(n, eps)

    n64 = nodes.astype(np.float64)
    p64 = pair.astype(np.float64)
    z = norm_rows(n64, 1e-12)
    zp = norm_rows(p64, 1e-12)
    zn = norm_rows(n64, 1e-8)
    logden = np.empty(N, dtype=np.float64)
    for i in range(0, N, 1024):
        sim = zn[i: i + 1024] @ zn.T
        den = (~mask_b[i: i + 1024] * np.exp(sim / T)).sum(1)
        logden[i: i + 1024] = np.log(den)
    loss = logden.sum() - float(z.sum(0) @ zp.sum(0)) / (N * T)
    return np.float32(loss)


def _run_eye(nodes, pair):
    global LAST_EXEC_TIME_NS
    nc = _program(False)

    # row n = g*1024 + p*8 + t  ->  arr[g][p, t, :]
    arr = nodes.reshape(8, 128, NB, D)
    parr = pair.reshape(8, 128, NB, D)
    in_maps = []
    for c in range(NCORES):
        m = {f"g{j}": np.ascontiguousarray(arr[(c + j) % 8]) for j in range(4)}
        m["gq"] = np.ascontiguousarray(arr[c + 4] if c < 4 else arr[c])
        m["gp"] = np.ascontiguousarray(
            arr[c][:, 0:4] if c < 4 else arr[c - 4][:, 4:8]
        )
        m["pr"] = np.ascontiguousarray(parr[c])
        in_maps.append(m)

    trace = bool(os.environ.get("BASS_TRACE"))
    if trace:
        _install_trace_hook()
    res = run_bass_kernel_spmd(nc, in_maps, list(range(NCORES)), trace=trace)
    LAST_EXEC_TIME_NS = res.exec_time_ns

    den_rows = np.zeros(N, dtype=np.float64)
    q = np.arange(1024)
    perm = (q % 128) * 8 + q // 128        # znT col q -> row offset in group
    S1 = np.zeros(D, dtype=np.float64)
    S2 = np.zeros(D, dtype=np.float64)
    for c in range(NCORES):
        r = res.results[c]
        rs = r["den"].astype(np.float64)                     # [128, 36]
        # diag + k1..k3 rowsums all target own-group rows m*8 + s
        own = rs[:, 0:32].reshape(128, 4, NB).sum(axis=1)    # [m, s]
        den_rows[c * R:(c + 1) * R] += own.reshape(-1)
        # pair-block rowsums: strips s -> t = s (c<4) or s+4 (c>=4)
        pbase = (c if c < 4 else c - 4) * R
        toff = 0 if c < 4 else 4
        pr_ = np.zeros((128, NB))
        pr_[:, toff:toff + 4] = rs[:, 32:36]
        den_rows[pbase:pbase + R] += pr_.reshape(-1)

        col = r["col"].astype(np.float64).reshape(-1)        # [4992]
        den_rows[c * R + perm[128:1024]] += col[0:896]
        for j in (1, 2, 3):
            g = (c + j) % 8
            den_rows[g * R + perm] += col[896 + 1024 * (j - 1): 896 + 1024 * j]
        gq = (c + 4) % 8 if c < 4 else c
        den_rows[gq * R + perm] += col[3968:4992]

        S1 += r["s1p"].astype(np.float64).reshape(NB, D).sum(0)
        S2 += r["s2p"].astype(np.float64).reshape(NB, D).sum(0)

    # exact self-term: device computed exp(sum_d znbf[u,d]^2 / T) with
    # bf16 zn operands and f32 accumulation; reproduce on host
    n64 = nodes.astype(np.float64)
    znb = n64 / np.linalg.norm(n64, axis=1, keepdims=True)
    znb16 = znb.astype(ml_dtypes.bfloat16).astype(np.float64)
    simuu = (znb16 * znb16).sum(1)
    den_rows -= np.exp(simuu / T)

    loss = np.log(den_rows).sum() - float(S1 @ S2) / (N * T)
    return np.float32(loss)


def _run_general(nodes, pair, mask_bf):
    global LAST_EXEC_TIME_NS
    nc = _program(True)

    in_maps = []
    for c in range(NCORES):
        sl = slice(c * R, (c + 1) * R)
        in_maps.append({
            "nodes_rm": nodes,
            "own_rm": np.ascontiguousarray(nodes[sl]),
            "pair_rm": np.ascontiguousarray(pair[sl]),
            "mask_bf": np.ascontiguousarray(mask_bf[sl]),
        })

    trace = bool(os.environ.get("BASS_TRACE"))
    if trace:
        _install_trace_hook()
    res = run_bass_kernel_spmd(nc, in_maps, list(range(NCORES)), trace=trace)
    LAST_EXEC_TIME_NS = res.exec_time_ns

    den_rows = np.empty(N, dtype=np.float64)
    S1 = np.zeros(D, dtype=np.float64)
    S2 = np.zeros(D, dtype=np.float64)
    for c in range(NCORES):
        r = res.results[c]
        den_pb = r["den"].astype(np.float64).reshape(128, NB, 4).sum(-1)
        den_pb -= r["corr"].astype(np.float64)
        # row j = c*1024 + b*128 + p  ->  den_pb[p, b]
        den_rows[c * R:(c + 1) * R] = den_pb.T.reshape(R)
        S1 += r["s1p"].astype(np.float64).reshape(NB, D).sum(0)
        S2 += r["s2p"].astype(np.float64).reshape(NB, D).sum(0)

    loss = np.log(den_rows).sum() - float(S1 @ S2) / (N * T)
    return np.float32(loss)


# revision 11
# speedup vs baseline: 1.4705x; 1.1436x over previous
"""Trainium2 Bass kernel for the NT-Xent style contrastive loss.

loss = sum_j log(den_sum[j]) - (S1 . S2) / (N*T)
  den_sum[j] = sum_k (~mask[j,k]) * exp(sim(zn_j, zn_k) / T)
  S1 = sum_i z_i,  S2 = sum_j z_p_j   (z / zn / z_p row-L2-normalized)

Eye-mask fast path exploits the SYMMETRY of E = exp(zn zn^T / T): only the
upper-triangle 1024x1024 blocks of the 8192x8192 matrix are exponentiated.
Row sums come free from the ScalarE activation accum_out; column sums of
each block (the mirrored rows' contributions) are ones-matmuls on the PE.

Work is balanced 36 strip-activations per core (33280 exp-columns, vs
65536 for the non-symmetric version):
  - diag block (c,c): triangle strips t: cols [t*128, 1024), rowsum via
    accum; colsum over cols [(t+1)*128, 1024) (excludes own 128-subtile
    whose mirrors are computed directly).
  - cyclic blocks (c, c+k) k=1..3: 8 full strips each, rowsum+colsum.
  - distance-4 pair {c, c+-4}: split by row halves between its two
    endpoint cores via dedicated gP (lhsT rows) / gQ (rhs cols) input
    slots; the host feeds core c>=4 the OTHER half's t-slices so the
    SPMD program stays uniform.
Both matmul operands are bf16 zn (rows pre-normalized), so the exp scale
is the constant 1/T*... = 2.0 and any core can run any strip. The host
combines row/col partials in f64 and subtracts the exact self-term.
"""

import os
import sys
import types
from contextlib import ExitStack

import numpy as np

sys.path.insert(0, "/opt/trn_rl_repo")

import ml_dtypes  # noqa: E402

import concourse.bass as bass  # noqa: E402
import concourse.tile as tile  # noqa: E402
from concourse import bacc, mybir  # noqa: E402
from concourse.bass_utils import run_bass_kernel_spmd  # noqa: E402
from concourse.masks import make_identity  # noqa: E402

# Route both Ln and Exp activations to the combined
# natural_log_exp_and_others table set: the default chooser picks the
# first set containing each function, which would force a ~2.7us table
# switch between the rsqrt prologue (exp(-0.5*ln(x))) and the main Exp
# stream. Blank the single-function sets (positions preserved, ids stay
# valid act_info.json indices) so only the combined set can serve them.
_GAT_ORIG = None


def _patch_act_tables():
    global _GAT_ORIG
    if _GAT_ORIG is not None:
        return
    import concourse.bacc as bacc_mod

    _GAT_ORIG = bacc_mod.get_activation_tables

    def patched(arch):
        t = _GAT_ORIG(arch)
        out = {}
        for name, fns in t.items():
            fns = set(fns)
            if name in ("exp_and_others", "exp_and_friends"):
                fns.discard(mybir.ActivationFunctionType.Exp)
            if name == "natural_log":
                fns.discard(mybir.ActivationFunctionType.Ln)
            out[name] = fns
        return out

    bacc_mod.get_activation_tables = patched

N = 8192
D = 128
NCORES = 8
T = 0.5
R = N // NCORES        # rows per core
NB = R // 128          # i-blocks (strips) per 1024-row group
F32 = mybir.dt.float32
BF16 = mybir.dt.bfloat16
AX = mybir.AxisListType
ALU = mybir.AluOpType
ACTF = mybir.ActivationFunctionType

# rsqrt seed: 1/sqrt(x) ~= A/x + B, minimax on x in [30, 400]
RSQ_A = 4.715
RSQ_B = 0.043133

LAST_EXEC_TIME_NS = None


def _install_trace_hook():
    """Make run_bass_kernel_spmd(trace=True) work under axon by supplying
    the antenv.axon_hooks module this image lacks."""
    try:
        if "antenv.axon_hooks" in sys.modules:
            return
        import antenv
        from trn_agent_boot.trn_boot import _ntff_profile_via_ctypes

        hook = _ntff_profile_via_ctypes("/opt/axon/libaxon_pjrt.so")
        m = types.ModuleType("antenv.axon_hooks")
        box = [hook]
        m.set_axon_ntff_profile_hook = lambda h: box.__setitem__(0, h)
        m.get_axon_ntff_profile_hook = lambda: box[0]
        sys.modules["antenv.axon_hooks"] = m
        antenv.axon_hooks = m
    except Exception:
        pass


def _bcast_inner(ap, n):
    """Broadcast a [P, F] AP to [P, F, n] with stride-0 innermost dim."""
    return bass.AP(tensor=ap.tensor, offset=ap.offset, ap=[*ap.ap, [0, n]])


def _newton_rsqrt(nc, pool, out, x, w):
    """out = 1/sqrt(x) elementwise, [128, w] f32, entirely on DVE.

    Seed A/x + B (~10% rel err on x in [30, 400]), then two Newton steps
    via scalar_tensor_tensor, whose (b - 1.5)*y form flips the sign each
    step; after an even number of steps the result is positive."""
    r = pool.tile([128, w], F32, tag="nt_r", name="nt_r")
    nc.vector.reciprocal(r, x)
    y0 = pool.tile([128, w], F32, tag="nt_y0", name="nt_y0")
    nc.vector.tensor_scalar(
        out=y0, in0=r, scalar1=RSQ_A, scalar2=RSQ_B, op0=ALU.mult, op1=ALU.add
    )
    xh = pool.tile([128, w], F32, tag="nt_xh", name="nt_xh")
    nc.vector.tensor_scalar_mul(xh, x, 0.5)
    y = y0
    for it in range(2):
        a = pool.tile([128, w], F32, tag="nt_a", name="nt_a")
        nc.vector.tensor_mul(a, y, y)
        b = pool.tile([128, w], F32, tag="nt_b", name="nt_b")
        nc.vector.tensor_mul(b, a, xh)
        y2 = out if it == 1 else pool.tile([128, w], F32, tag="nt_y", name="nt_y")
        nc.vector.scalar_tensor_tensor(
            out=y2, in0=b, scalar=1.5, in1=y, op0=ALU.subtract, op1=ALU.mult
        )
        y = y2
    return out


def _split512(lo, hi):
    """Split [lo, hi) at multiples of 512 (PSUM bank boundaries)."""
    out = []
    a = lo
    while a < hi:
        b = min((a // 512 + 1) * 512, hi)
        out.append((a, b))
        a = b
    return out


# column layout within the packed norm tiles [128, 52]:
# g0 [0:8) g1 [8:16) g2 [16:24) g3 [24:32) Q [32:40) P [40:44) pair [44:52)
_NCOL = {"g0": (0, 8), "g1": (8, 16), "g2": (16, 24), "g3": (24, 32),
         "gq": (32, 40), "gp": (40, 44), "pr": (44, 52)}

# colsum dram layout: diag 896 | k1 1024 | k2 1024 | k3 1024 | P 1024
_COL_OFF = {"diag": 0, "k1": 896, "k2": 1920, "k3": 2944, "p": 3968}


def _build_sym():
    _patch_act_tables()
    nc = bacc.Bacc(
        "TRN2", target_bir_lowering=False, debug=False, num_devices=NCORES
    )
    g_in = [
        nc.dram_tensor(f"g{j}", [128, NB, D], F32, kind="ExternalInput").ap()
        for j in range(4)
    ]
    gq_in = nc.dram_tensor("gq", [128, NB, D], F32, kind="ExternalInput").ap()
    gp_in = nc.dram_tensor("gp", [128, 4, D], F32, kind="ExternalInput").ap()
    pr_in = nc.dram_tensor("pr", [128, NB, D], F32, kind="ExternalInput").ap()
    den_out = nc.dram_tensor("den", [128, 36], F32, kind="ExternalOutput").ap()
    col_out = nc.dram_tensor("col", [1, 4992], F32, kind="ExternalOutput").ap()
    s1_out = nc.dram_tensor("s1p", [1, R], F32, kind="ExternalOutput").ap()
    s2_out = nc.dram_tensor("s2p", [1, R], F32, kind="ExternalOutput").ap()

    with tile.TileContext(nc) as tc, ExitStack() as ctx:
        pers = ctx.enter_context(tc.tile_pool(name="pers", bufs=1))
        spool = ctx.enter_context(tc.tile_pool(name="spool", bufs=2))
        epool = ctx.enter_context(tc.tile_pool(name="epool", bufs=4))
        depool = ctx.enter_context(tc.tile_pool(name="depool", bufs=8))
        pmm = ctx.enter_context(tc.tile_pool(name="pmm", bufs=2, space="PSUM"))
        pcol = ctx.enter_context(tc.tile_pool(name="pcol", bufs=1, space="PSUM"))
        ptx = ctx.enter_context(tc.tile_pool(name="ptx", bufs=2, space="PSUM"))

        ident = pers.tile([128, 128], BF16)
        ones_bf = pers.tile([128, 1], BF16)
        den_sb = pers.tile([128, 36], F32)
        n2 = pers.tile([128, 52], F32)
        n2c = pers.tile([128, 52], F32)
        lns = pers.tile([128, 52], F32)
        inv = pers.tile([128, 52], F32)
        col_sb = pers.tile([1, 4992], F32)
        s_sb = pers.tile([1, 2 * R], F32)

        rm = {}
        for nm, ap_in, nt in (
            ("g0", g_in[0], NB), ("g1", g_in[1], NB), ("g2", g_in[2], NB),
            ("g3", g_in[3], NB), ("gq", gq_in, NB), ("gp", gp_in, 4),
            ("pr", pr_in, NB),
        ):
            rm[nm] = pers.tile([128, nt, D], F32, name=f"rm_{nm}")
        zn = {nm: pers.tile([128, nt, D], BF16, name=f"zn_{nm}")
              for nm, nt in (("g0", NB), ("g1", NB), ("g2", NB), ("g3", NB),
                             ("gq", NB), ("gp", 4))}
        znT = {nm: pers.tile([128, nt * 128], BF16, name=f"znT_{nm}")
               for nm, nt in (("g0", NB), ("g1", NB), ("g2", NB), ("g3", NB),
                              ("gq", NB), ("gp", 4))}

        # ---- input DMAs: g0/g1 race in parallel on separate rings so both
        # are resident before the diag block needs them
        nc.sync.dma_start(out=rm["g0"], in_=g_in[0])
        nc.sync.dma_start(out=rm["g2"], in_=g_in[2])
        nc.sync.dma_start(out=rm["g3"], in_=g_in[3])
        nc.scalar.dma_start(out=rm["g1"], in_=g_in[1])
        nc.scalar.dma_start(out=rm["gq"], in_=gq_in)
        nc.scalar.dma_start(out=rm["gp"], in_=gp_in)
        nc.scalar.dma_start(out=rm["pr"], in_=pr_in)

        make_identity(nc, ident)
        nc.vector.memset(ones_bf, 1.0)
        # warm the (combined ln+exp) table while inputs are still loading
        junk1 = pers.tile([128, 1], F32)
        nc.scalar.activation(out=junk1, in_=ones_bf, func=ACTF.Exp)

        # ---- norm machinery: 1/sqrt(n2) = exp(-0.5 * ln(n2)) on ScalarE
        # (same table set as the main Exp stream -> zero table switches),
        # squares on DVE (critical groups) or GPSIMD, reduces on DVE.
        def norm_sq(nm, tlo, thi, eng):
            a0 = _NCOL[nm][0]
            sq = spool.tile(
                [128, thi - tlo, D], F32, tag="sq", name=f"sq_{nm}{tlo}"
            )
            eng.tensor_mul(sq, rm[nm][:, tlo:thi, :], rm[nm][:, tlo:thi, :])
            nc.vector.tensor_reduce(
                out=n2[:, a0 + tlo: a0 + thi], in_=sq, axis=AX.X, op=ALU.add
            )

        def norm_inv(a, b):
            nc.vector.tensor_scalar_max(n2c[:, a:b], n2[:, a:b], 30.0)
            nc.scalar.activation(out=lns[:, a:b], in_=n2c[:, a:b], func=ACTF.Ln)
            nc.scalar.activation(
                out=inv[:, a:b], in_=lns[:, a:b], func=ACTF.Exp, scale=-0.5
            )

        def zn_mul(nm, tlo=0, thi=None):
            a0 = _NCOL[nm][0]
            if thi is None:
                thi = rm[nm].shape[1]
            nc.vector.tensor_mul(
                zn[nm][:, tlo:thi, :], rm[nm][:, tlo:thi, :],
                _bcast_inner(inv[:, a0 + tlo: a0 + thi], D),
            )

        def transposes(nm, order):
            nt = zn[nm].shape[1]
            pst = ptx.tile([128, nt, 128], BF16, tag="trx", name=f"pst_{nm}")
            for t in order:
                nc.tensor.transpose(pst[:, t, :], zn[nm][:, t, :], ident)
            return pst

        # g0 norms, upper half (rows t=4..7) first: the diag block walks
        # strips widest-last so those rows' operands are needed first
        norm_sq("g0", 4, NB, nc.vector)
        norm_inv(4, 8)
        norm_sq("g0", 0, 4, nc.vector)
        norm_inv(0, 4)
        # g1 squares fire on GPSIMD in parallel; reduces are deferred below
        # so they only backfill DVE idle slots (tile scheduling is
        # priority==emission-order among *ready* ops)
        sq1a = spool.tile([128, 4, D], F32, tag="sq1", name="sq1a")
        nc.gpsimd.tensor_mul(sq1a, rm["g1"][:, 0:4, :], rm["g1"][:, 0:4, :])
        sq1b = spool.tile([128, 4, D], F32, tag="sq1", name="sq1b")
        nc.gpsimd.tensor_mul(sq1b, rm["g1"][:, 4:NB, :], rm["g1"][:, 4:NB, :])

        # ---- strip machinery
        def strip(lhsT_src, lt, rhs_src, lo, hi, slot, epool_, etag):
            w = hi - lo
            p = pmm.tile([128, w], F32, tag="ps", name=f"ps_{slot}")
            for a, b in _split512(0, w):
                nc.tensor.matmul(
                    out=p[:, a:b],
                    lhsT=lhsT_src[:, lt * 128:(lt + 1) * 128],
                    rhs=rhs_src[:, lo + a: lo + b],
                    start=True,
                    stop=True,
                )
            e = epool_.tile([128, w], BF16, tag=etag, name=f"e_{slot}")
            nc.scalar.activation(
                out=e, in_=p, func=ACTF.Exp, scale=1.0 / T,
                accum_out=den_sb[:, slot:slot + 1],
            )
            return e

        def colsum_mms(ctile, e, e_off, lo, hi, first_banks, last_banks):
            """ctile[lo:hi) += ones^T @ e[:, e_off + (.-lo)], bank-aware flags.
            first_banks/last_banks: sets of bank indices for which this is
            the first / last accumulating matmul."""
            for a, b in _split512(lo, hi):
                bank = a // 512
                nc.tensor.matmul(
                    out=ctile[:, a:b],
                    lhsT=ones_bf,
                    rhs=e[:, e_off + a - lo: e_off + b - lo],
                    start=bank in first_banks,
                    stop=bank in last_banks,
                )

        # ---- diag block: triangle strips widest-last, zn/transpose/copy
        # emitted per-tile right ahead of each strip; E retained (depool)
        # for the colsum pass that runs between k1 and k2
        pst0 = ptx.tile([128, NB, 128], BF16, tag="trx", name="pst_g0")
        diag_e = {}
        for t in range(NB - 1, -1, -1):
            zn_mul("g0", t, t + 1)
            nc.tensor.transpose(pst0[:, t, :], zn["g0"][:, t, :], ident)
            nc.vector.tensor_copy(
                znT["g0"][:, t * 128:(t + 1) * 128], pst0[:, t, :]
            )
            diag_e[t] = strip(
                znT["g0"], t, znT["g0"], t * 128, 1024, t, depool, "de"
            )
            if t == 2:
                # g1 inv lands mid-diag: reduces backfill DVE gaps, the
                # ln/exp pair slots into the ScalarE act stream here
                for q in range(4):
                    nc.vector.tensor_reduce(
                        out=n2[:, 8 + 2 * q: 10 + 2 * q],
                        in_=(sq1a if q < 2 else sq1b)[:, 2 * (q % 2): 2 * (q % 2) + 2, :],
                        axis=AX.X, op=ALU.add,
                    )
                norm_inv(8, 16)

        zn_mul("g1")
        pst1 = transposes("g1", range(NB))
        nc.vector.tensor_copy(znT["g1"][:, 0:512], pst1[:, 0:4, :])
        nc.vector.tensor_copy(znT["g1"][:, 512:1024], pst1[:, 4:NB, :])

        # ---- cyclic blocks k=1..3 + pair block: one-ahead mm emission so
        # PE never in-order-stalls ScalarE
        def block(lhsT_src, rhs_src, slot0, nstrips, ckey, extra=None):
            ctile = pcol.tile([1, 1024], F32, tag="col", name=f"c_{ckey}")
            es = {}
            es[0] = strip(lhsT_src, 0, rhs_src, 0, 1024, slot0, epool, "e")
            for s in range(1, nstrips + 1):
                if s <= nstrips - 1:
                    es[s] = strip(
                        lhsT_src, s, rhs_src, 0, 1024, slot0 + s, epool, "e"
                    )
                if extra is not None and s - 1 in extra:
                    extra[s - 1]()
                colsum_mms(
                    ctile, es[s - 1], 0, 0, 1024,
                    first_banks={0, 1} if s - 1 == 0 else set(),
                    last_banks={0, 1} if s - 1 == nstrips - 1 else set(),
                )
                del es[s - 1]
            nc.vector.tensor_copy(
                col_sb[:, _COL_OFF[ckey]:_COL_OFF[ckey] + 1024], ctile
            )
            nc.gpsimd.dma_start(
                out=col_out[:, _COL_OFF[ckey]:_COL_OFF[ckey] + 1024],
                in_=col_sb[:, _COL_OFF[ckey]:_COL_OFF[ckey] + 1024],
            )

        def late_sq(nm):
            nt = rm[nm].shape[1]
            return lambda: norm_sq(nm, 0, nt, nc.gpsimd)

        def late_reds():
            # halves keep each backfilled DVE op small
            for nm in ("g2", "g3", "gq", "pr"):
                a0 = _NCOL[nm][0]
                for h in (0, 4):
                    sqh = spool.tile(
                        [128, 4, D], F32, tag="sq", name=f"lsq_{nm}{h}"
                    )
                    nc.gpsimd.tensor_mul(
                        sqh, rm[nm][:, h:h + 4, :], rm[nm][:, h:h + 4, :]
                    )
                    nc.vector.tensor_reduce(
                        out=n2[:, a0 + h: a0 + h + 4], in_=sqh,
                        axis=AX.X, op=ALU.add,
                    )
            sqp = spool.tile([128, 4, D], F32, tag="sq", name="lsq_gp")
            nc.gpsimd.tensor_mul(sqp, rm["gp"], rm["gp"])
            nc.vector.tensor_reduce(
                out=n2[:, 40:44], in_=sqp, axis=AX.X, op=ALU.add
            )

        block(znT["g0"], znT["g1"], 8, NB, "k1", extra={
            1: late_reds,
            4: lambda: norm_inv(16, 52),
            5: lambda: (zn_mul("g2"), zn_mul("gq"))[-1],
            6: lambda: nc.vector.tensor_copy(
                znT["g2"], transposes("g2", range(NB))
            ),
        })

        # ---- diag colsums: cols [128, 1024) of the group, ascending so
        # strip 0 (full width) opens both accumulation banks
        dcol = pcol.tile([1, 896], F32, tag="col", name="c_diag")
        for t in range(NB - 1):
            lo = t * 128
            first = {0, 1} if t == 0 else set()
            last = set()
            if t == 3:
                last.add(0)       # bank 0: writers t=0..3
            if t == NB - 2:
                last.add(1)       # bank 1: writers t=0..6
            colsum_mms(dcol, diag_e[t], 128, lo, 896, first, last)
        nc.vector.tensor_copy(col_sb[:, 0:896], dcol)
        nc.gpsimd.dma_start(out=col_out[:, 0:896], in_=col_sb[:, 0:896])

        # ---- S1/S2 partials via inv-weighted PE column sums of the raw
        # rows (no zn row-major tensors needed): S[t*128+d] partial
        # = sum_p inv[p,t] * rm[p,t,d]
        def s_mms(nm, off):
            a0 = _NCOL[nm][0]
            for half in (0, 1):
                sp = ptx.tile([1, 512], F32, tag="trx", name=f"s_{nm}{half}")
                for tt in range(4):
                    t = half * 4 + tt
                    nc.tensor.matmul(
                        out=sp[:, tt * 128:(tt + 1) * 128],
                        lhsT=inv[:, a0 + t: a0 + t + 1],
                        rhs=rm[nm][:, t, :],
                        start=tt == 0,
                        stop=tt == 3,
                    )
                nc.vector.tensor_copy(
                    s_sb[:, off + half * 512: off + (half + 1) * 512], sp
                )

        block(znT["g0"], znT["g2"], 16, NB, "k2", extra={
            2: lambda: zn_mul("g3"),
            4: lambda: nc.vector.tensor_copy(
                znT["g3"], transposes("g3", range(NB))
            ),
            6: lambda: (zn_mul("gp"), nc.vector.tensor_copy(
                znT["gq"], transposes("gq", range(NB))))[-1],
        })
        block(znT["g0"], znT["g3"], 24, NB, "k3", extra={
            2: lambda: nc.vector.tensor_copy(
                znT["gp"], transposes("gp", range(4))
            ),
            4: lambda: s_mms("g0", 0),
            6: lambda: s_mms("pr", R),
        })
        nc.gpsimd.dma_start(out=den_out[:, 0:32], in_=den_sb[:, 0:32])
        nc.gpsimd.dma_start(out=s1_out, in_=s_sb[:, 0:R])
        nc.gpsimd.dma_start(out=s2_out, in_=s_sb[:, R:2 * R])

        block(znT["gp"], znT["gq"], 32, 4, "p")
        nc.gpsimd.dma_start(out=den_out[:, 32:36], in_=den_sb[:, 32:36])

    nc.compile()
    return nc


def _build_general():
    """Correctness fallback for an arbitrary boolean mask (bf16 0/1 input).
    den correction per row: corr = sum_k mask[j,k] * E[j,k] via DVE
    tensor_tensor_reduce over the exp'd row block."""
    NCHG = 4
    CHG = N // NCHG
    nc = bacc.Bacc(
        "TRN2", target_bir_lowering=False, debug=False, num_devices=NCORES
    )
    nodes_rm = nc.dram_tensor("nodes_rm", [N, D], F32, kind="ExternalInput").ap()
    own_rm = nc.dram_tensor("own_rm", [R, D], F32, kind="ExternalInput").ap()
    pair_rm = nc.dram_tensor("pair_rm", [R, D], F32, kind="ExternalInput").ap()
    mask_bf = nc.dram_tensor("mask_bf", [R, N], BF16, kind="ExternalInput").ap()
    den_out = nc.dram_tensor("den", [128, NB * NCHG], F32, kind="ExternalOutput").ap()
    s1_out = nc.dram_tensor("s1p", [1, R], F32, kind="ExternalOutput").ap()
    s2_out = nc.dram_tensor("s2p", [1, R], F32, kind="ExternalOutput").ap()
    corr_out = nc.dram_tensor("corr", [128, NB], F32, kind="ExternalOutput").ap()

    NT = N // 128

    with tile.TileContext(nc) as tc, ExitStack() as ctx:
        persist = ctx.enter_context(tc.tile_pool(name="persist", bufs=1))
        znT = persist.tile([128, N], BF16)
        own_bf = persist.tile([128, R], BF16)
        inv_all = persist.tile([128, 80], F32)
        inv_ri_T = persist.tile([128, NB], F32)
        den_sb = persist.tile([128, NB, NCHG], F32)
        corr_sb = persist.tile([128, NB], F32)

        with (
            tc.tile_pool(name="pro", bufs=1) as pro,
            tc.tile_pool(name="psum_pro", bufs=1, space="PSUM") as psum_pro,
            tc.tile_pool(name="psum_tr", bufs=2, space="PSUM") as psum_tr,
        ):
            rm_sb = pro.tile([128, NT, D], F32)
            nc.sync.dma_start(
                out=rm_sb, in_=nodes_rm.rearrange("(t p) d -> p t d", p=128)
            )
            own_rm_sb = pro.tile([128, NB, D], F32)
            nc.sync.dma_start(
                out=own_rm_sb, in_=own_rm.rearrange("(t p) d -> p t d", p=128)
            )
            pair_rm_sb = pro.tile([128, NB, D], F32)
            nc.sync.dma_start(
                out=pair_rm_sb, in_=pair_rm.rearrange("(t p) d -> p t d", p=128)
            )

            ident = pro.tile([128, 128], BF16)
            make_identity(nc, ident)
            ones = pro.tile([128, 1], F32)
            nc.vector.memset(ones, 1.0)

            sq = pro.tile([128, NT, D], F32)
            nc.vector.tensor_mul(sq, rm_sb, rm_sb)
            norm2 = pro.tile([128, 80], F32)
            nc.vector.tensor_reduce(
                out=norm2[:, 0:NT], in_=sq, axis=AX.X, op=ALU.add
            )
            sq_own = pro.tile([128, NB, D], F32)
            nc.vector.tensor_mul(sq_own, own_rm_sb, own_rm_sb)
            nc.vector.tensor_reduce(
                out=norm2[:, NT: NT + NB], in_=sq_own, axis=AX.X, op=ALU.add
            )
            sq_pair = pro.tile([128, NB, D], F32)
            nc.vector.tensor_mul(sq_pair, pair_rm_sb, pair_rm_sb)
            nc.vector.tensor_reduce(
                out=norm2[:, NT + NB: NT + 2 * NB],
                in_=sq_pair,
                axis=AX.X,
                op=ALU.add,
            )
            norm2c = pro.tile([128, 80], F32)
            nc.vector.tensor_scalar_max(norm2c, norm2, 30.0)
            _newton_rsqrt(nc, pro, inv_all, norm2c, 80)
            inv_r_pt = inv_all[:, 0:NT]
            inv_ri = inv_all[:, NT: NT + NB]
            inv_rp = inv_all[:, NT + NB: NT + 2 * NB]

            nc.vector.tensor_scalar_mul(inv_ri_T, inv_ri, 1.0 / T)

            zn_rm = pro.tile([128, NT, D], BF16)
            nc.vector.tensor_mul(zn_rm, rm_sb, _bcast_inner(inv_r_pt, D))
            own_rm_bf = pro.tile([128, NB, D], BF16)
            nc.vector.tensor_copy(own_rm_bf, own_rm_sb)

            for g in range(NT // NB):
                pst = psum_tr.tile([128, NB, 128], BF16)
                for t in range(NB):
                    nc.tensor.transpose(
                        pst[:, t, :], zn_rm[:, g * NB + t, :], ident
                    )
                nc.vector.tensor_copy(
                    znT[:, g * NB * 128:(g + 1) * NB * 128], pst
                )
            pst_o = psum_tr.tile([128, NB, 128], BF16)
            for t in range(NB):
                nc.tensor.transpose(pst_o[:, t, :], own_rm_bf[:, t, :], ident)
            nc.vector.tensor_copy(own_bf, pst_o)

            zsc = pro.tile([128, NB, D], F32)
            nc.vector.tensor_mul(zsc, own_rm_sb, _bcast_inner(inv_ri, D))
            zpsc = pro.tile([128, NB, D], F32)
            nc.vector.tensor_mul(zpsc, pair_rm_sb, _bcast_inner(inv_rp, D))
            s1ps = psum_pro.tile([1, R], F32)
            s2ps = psum_pro.tile([1, R], F32)
            zsc_f = zsc.rearrange("p t d -> p (t d)")
            zpsc_f = zpsc.rearrange("p t d -> p (t d)")
            for h in range(R // 512):
                nc.tensor.matmul(
                    out=s1ps[:, h * 512:(h + 1) * 512],
                    lhsT=ones,
                    rhs=zsc_f[:, h * 512:(h + 1) * 512],
                    start=True,
                    stop=True,
                )
                nc.tensor.matmul(
                    out=s2ps[:, h * 512:(h + 1) * 512],
                    lhsT=ones,
                    rhs=zpsc_f[:, h * 512:(h + 1) * 512],
                    start=True,
                    stop=True,
                )
            s1sb = pro.tile([1, R], F32)
            nc.vector.tensor_copy(s1sb, s1ps)
            s2sb = pro.tile([1, R], F32)
            nc.vector.tensor_copy(s2sb, s2ps)
            nc.sync.dma_start(out=s1_out, in_=s1sb)
            nc.sync.dma_start(out=s2_out, in_=s2sb)

        with (
            tc.tile_pool(name="psum_main", bufs=2, space="PSUM") as psum_main,
            tc.tile_pool(name="erow", bufs=2) as epool,
            tc.tile_pool(name="mrow", bufs=2) as mpool,
            tc.tile_pool(name="tjunk", bufs=2) as tjpool,
        ):
            for b in range(NB):
                erow = epool.tile([128, N], BF16)
                mrow = mpool.tile([128, N], BF16)
                nc.sync.dma_start(
                    out=mrow, in_=mask_bf[b * 128:(b + 1) * 128, :]
                )
                for chi in range(NCHG):
                    p = psum_main.tile([128, CHG], F32)
                    for j in range(CHG // 512):
                        k0 = chi * CHG + j * 512
                        nc.tensor.matmul(
                            out=p[:, j * 512:(j + 1) * 512],
                            lhsT=own_bf[:, b * 128:(b + 1) * 128],
                            rhs=znT[:, k0: k0 + 512],
                            start=True,
                            stop=True,
                        )
                    nc.scalar.activation(
                        out=erow[:, chi * CHG:(chi + 1) * CHG],
                        in_=p,
                        func=ACTF.Exp,
                        scale=inv_ri_T[:, b: b + 1],
                        accum_out=den_sb[:, b, chi: chi + 1],
                    )
                tj = tjpool.tile([128, N], BF16)
                nc.vector.tensor_tensor_reduce(
                    out=tj,
                    in0=erow,
                    in1=mrow,
                    scale=1.0,
                    scalar=0.0,
                    op0=ALU.mult,
                    op1=ALU.add,
                    accum_out=corr_sb[:, b: b + 1],
                )
            nc.sync.dma_start(out=den_out, in_=den_sb)
            nc.sync.dma_start(out=corr_out, in_=corr_sb)

    nc.compile()
    return nc


_PROGRAMS = {}


def _program(general: bool):
    if general not in _PROGRAMS:
        _PROGRAMS[general] = _build_general() if general else _build_sym()
    return _PROGRAMS[general]


def kernel(nodes, pair_nodes, nodes_labels, mask):
    global LAST_EXEC_TIME_NS
    nodes = np.ascontiguousarray(np.asarray(nodes), dtype=np.float32)
    pair = np.ascontiguousarray(np.asarray(pair_nodes), dtype=np.float32)
    mask = np.asarray(mask)
    assert nodes.shape == (N, D) and pair.shape == (N, D)

    mask_b = mask.astype(bool, copy=False)
    is_eye = bool(np.count_nonzero(mask_b) == N) and bool(
        mask_b.diagonal().all()
    )

    if not is_eye:
        try:
            mask_bf = mask_b.astype(ml_dtypes.bfloat16)
            return _run_general(nodes, pair, mask_bf)
        except Exception:
            return _host_fallback(nodes, pair, mask_b)
    return _run_eye(nodes, pair)


def _host_fallback(nodes, pair, mask_b):
    """Numpy reference for masks the device fallback cannot handle."""
    def norm_rows(x, eps):
        n = np.linalg.norm(x, axis=1, keepdims=True)
        return x / np.maximum(n, eps)

    n64 = nodes.astype(np.float64)
    p64 = pair.astype(np.float64)
    z = norm_rows(n64, 1e-12)
    zp = norm_rows(p64, 1e-12)
    zn = norm_rows(n64, 1e-8)
    logden = np.empty(N, dtype=np.float64)
    for i in range(0, N, 1024):
        sim = zn[i: i + 1024] @ zn.T
        den = (~mask_b[i: i + 1024] * np.exp(sim / T)).sum(1)
        logden[i: i + 1024] = np.log(den)
    loss = logden.sum() - float(z.sum(0) @ zp.sum(0)) / (N * T)
    return np.float32(loss)


def _run_eye(nodes, pair):
    global LAST_EXEC_TIME_NS
    nc = _program(False)

    # row n = g*1024 + p*8 + t  ->  arr[g][p, t, :]
    arr = nodes.reshape(8, 128, NB, D)
    parr = pair.reshape(8, 128, NB, D)
    in_maps = []
    for c in range(NCORES):
        m = {f"g{j}": np.ascontiguousarray(arr[(c + j) % 8]) for j in range(4)}
        m["gq"] = np.ascontiguousarray(arr[c + 4] if c < 4 else arr[c])
        m["gp"] = np.ascontiguousarray(
            arr[c][:, 0:4] if c < 4 else arr[c - 4][:, 4:8]
        )
        m["pr"] = np.ascontiguousarray(parr[c])
        in_maps.append(m)

    trace = bool(os.environ.get("BASS_TRACE"))
    if trace:
        _install_trace_hook()
    res = run_bass_kernel_spmd(nc, in_maps, list(range(NCORES)), trace=trace)
    LAST_EXEC_TIME_NS = res.exec_time_ns

    den_rows = np.zeros(N, dtype=np.float64)
    q = np.arange(1024)
    perm = (q % 128) * 8 + q // 128        # znT col q -> row offset in group
    S1 = np.zeros(D, dtype=np.float64)
    S2 = np.zeros(D, dtype=np.float64)
    for c in range(NCORES):
        r = res.results[c]
        rs = r["den"].astype(np.float64)                     # [128, 36]
        # diag + k1..k3 rowsums all target own-group rows m*8 + s
        own = rs[:, 0:32].reshape(128, 4, NB).sum(axis=1)    # [m, s]
        den_rows[c * R:(c + 1) * R] += own.reshape(-1)
        # pair-block rowsums: strips s -> t = s (c<4) or s+4 (c>=4)
        pbase = (c if c < 4 else c - 4) * R
        toff = 0 if c < 4 else 4
        pr_ = np.zeros((128, NB))
        pr_[:, toff:toff + 4] = rs[:, 32:36]
        den_rows[pbase:pbase + R] += pr_.reshape(-1)

        col = r["col"].astype(np.float64).reshape(-1)        # [4992]
        den_rows[c * R + perm[128:1024]] += col[0:896]
        for j in (1, 2, 3):
            g = (c + j) % 8
            den_rows[g * R + perm] += col[896 + 1024 * (j - 1): 896 + 1024 * j]
        gq = (c + 4) % 8 if c < 4 else c
        den_rows[gq * R + perm] += col[3968:4992]

        S1 += r["s1p"].astype(np.float64).reshape(NB, D).sum(0)
        S2 += r["s2p"].astype(np.float64).reshape(NB, D).sum(0)

    # exact self-term: device computed exp(sum_d znbf[u,d]^2 / T) with
    # bf16 zn operands and f32 accumulation; reproduce on host
    n64 = nodes.astype(np.float64)
    znb = n64 / np.linalg.norm(n64, axis=1, keepdims=True)
    znb16 = znb.astype(ml_dtypes.bfloat16).astype(np.float64)
    simuu = (znb16 * znb16).sum(1)
    den_rows -= np.exp(simuu / T)

    loss = np.log(den_rows).sum() - float(S1 @ S2) / (N * T)
    return np.float32(loss)


def _run_general(nodes, pair, mask_bf):
    global LAST_EXEC_TIME_NS
    nc = _program(True)

    in_maps = []
    for c in range(NCORES):
        sl = slice(c * R, (c + 1) * R)
        in_maps.append({
            "nodes_rm": nodes,
            "own_rm": np.ascontiguousarray(nodes[sl]),
            "pair_rm": np.ascontiguousarray(pair[sl]),
            "mask_bf": np.ascontiguousarray(mask_bf[sl]),
        })

    trace = bool(os.environ.get("BASS_TRACE"))
    if trace:
        _install_trace_hook()
    res = run_bass_kernel_spmd(nc, in_maps, list(range(NCORES)), trace=trace)
    LAST_EXEC_TIME_NS = res.exec_time_ns

    den_rows = np.empty(N, dtype=np.float64)
    S1 = np.zeros(D, dtype=np.float64)
    S2 = np.zeros(D, dtype=np.float64)
    for c in range(NCORES):
        r = res.results[c]
        den_pb = r["den"].astype(np.float64).reshape(128, NB, 4).sum(-1)
        den_pb -= r["corr"].astype(np.float64)
        # row j = c*1024 + b*128 + p  ->  den_pb[p, b]
        den_rows[c * R:(c + 1) * R] = den_pb.T.reshape(R)
        S1 += r["s1p"].astype(np.float64).reshape(NB, D).sum(0)
        S2 += r["s2p"].astype(np.float64).reshape(NB, D).sum(0)

    loss = np.log(den_rows).sum() - float(S1 @ S2) / (N * T)
    return np.float32(loss)


# revision 17
# speedup vs baseline: 1.5391x; 1.0466x over previous
"""Trainium2 Bass kernel for the NT-Xent style contrastive loss.

loss = sum_j log(den_sum[j]) - (S1 . S2) / (N*T)
  den_sum[j] = sum_k (~mask[j,k]) * exp(sim(zn_j, zn_k) / T)
  S1 = sum_i z_i,  S2 = sum_j z_p_j   (z / zn / z_p row-L2-normalized)

Eye-mask fast path exploits the SYMMETRY of E = exp(zn zn^T / T): only the
upper-triangle 1024x1024 blocks of the 8192x8192 matrix are exponentiated.
Row sums come free from the ScalarE activation accum_out; column sums of
each block (the mirrored rows' contributions) are ones-matmuls on the PE.

Work is balanced 36 strip-activations per core (33280 exp-columns, vs
65536 for the non-symmetric version):
  - diag block (c,c): triangle strips t: cols [t*128, 1024), rowsum via
    accum; colsum over cols [(t+1)*128, 1024) (excludes own 128-subtile
    whose mirrors are computed directly).
  - cyclic blocks (c, c+k) k=1..3: 8 full strips each, rowsum+colsum.
  - distance-4 pair {c, c+-4}: split by row halves between its two
    endpoint cores via dedicated gP (lhsT rows) / gQ (rhs cols) input
    slots; the host feeds core c>=4 the OTHER half's t-slices so the
    SPMD program stays uniform.
Both matmul operands are bf16 zn (rows pre-normalized), so the exp scale
is the constant 1/T*... = 2.0 and any core can run any strip. The host
combines row/col partials in f64 and subtracts the exact self-term.
"""

import os
import sys
import types
from contextlib import ExitStack

import numpy as np

sys.path.insert(0, "/opt/trn_rl_repo")

import ml_dtypes  # noqa: E402

import concourse.bass as bass  # noqa: E402
import concourse.tile as tile  # noqa: E402
from concourse import bacc, mybir  # noqa: E402
from concourse.bass_utils import run_bass_kernel_spmd  # noqa: E402
from concourse.masks import make_identity  # noqa: E402

# Route both Ln and Exp activations to the combined
# natural_log_exp_and_others table set: the default chooser picks the
# first set containing each function, which would force a ~2.7us table
# switch between the rsqrt prologue (exp(-0.5*ln(x))) and the main Exp
# stream. Blank the single-function sets (positions preserved, ids stay
# valid act_info.json indices) so only the combined set can serve them.
_GAT_ORIG = None


def _patch_act_tables():
    global _GAT_ORIG
    if _GAT_ORIG is not None:
        return
    import concourse.bacc as bacc_mod

    _GAT_ORIG = bacc_mod.get_activation_tables

    def patched(arch):
        t = _GAT_ORIG(arch)
        out = {}
        for name, fns in t.items():
            fns = set(fns)
            if name in ("exp_and_others", "exp_and_friends"):
                fns.discard(mybir.ActivationFunctionType.Exp)
            if name == "natural_log":
                fns.discard(mybir.ActivationFunctionType.Ln)
            out[name] = fns
        return out

    bacc_mod.get_activation_tables = patched

N = 8192
D = 128
NCORES = 8
T = 0.5
R = N // NCORES        # rows per core
NB = R // 128          # i-blocks (strips) per 1024-row group
F32 = mybir.dt.float32
BF16 = mybir.dt.bfloat16
AX = mybir.AxisListType
ALU = mybir.AluOpType
ACTF = mybir.ActivationFunctionType

# rsqrt seed: 1/sqrt(x) ~= A/x + B, minimax on x in [30, 400]
RSQ_A = 4.715
RSQ_B = 0.043133

LAST_EXEC_TIME_NS = None


def _install_trace_hook():
    """Make run_bass_kernel_spmd(trace=True) work under axon by supplying
    the antenv.axon_hooks module this image lacks."""
    try:
        if "antenv.axon_hooks" in sys.modules:
            return
        import antenv
        from trn_agent_boot.trn_boot import _ntff_profile_via_ctypes

        hook = _ntff_profile_via_ctypes("/opt/axon/libaxon_pjrt.so")
        m = types.ModuleType("antenv.axon_hooks")
        box = [hook]
        m.set_axon_ntff_profile_hook = lambda h: box.__setitem__(0, h)
        m.get_axon_ntff_profile_hook = lambda: box[0]
        sys.modules["antenv.axon_hooks"] = m
        antenv.axon_hooks = m
    except Exception:
        pass


def _bcast_inner(ap, n):
    """Broadcast a [P, F] AP to [P, F, n] with stride-0 innermost dim."""
    return bass.AP(tensor=ap.tensor, offset=ap.offset, ap=[*ap.ap, [0, n]])


def _newton_rsqrt(nc, pool, out, x, w):
    """out = 1/sqrt(x) elementwise, [128, w] f32, entirely on DVE.

    Seed A/x + B (~10% rel err on x in [30, 400]), then two Newton steps
    via scalar_tensor_tensor, whose (b - 1.5)*y form flips the sign each
    step; after an even number of steps the result is positive."""
    r = pool.tile([128, w], F32, tag="nt_r", name="nt_r")
    nc.vector.reciprocal(r, x)
    y0 = pool.tile([128, w], F32, tag="nt_y0", name="nt_y0")
    nc.vector.tensor_scalar(
        out=y0, in0=r, scalar1=RSQ_A, scalar2=RSQ_B, op0=ALU.mult, op1=ALU.add
    )
    xh = pool.tile([128, w], F32, tag="nt_xh", name="nt_xh")
    nc.vector.tensor_scalar_mul(xh, x, 0.5)
    y = y0
    for it in range(2):
        a = pool.tile([128, w], F32, tag="nt_a", name="nt_a")
        nc.vector.tensor_mul(a, y, y)
        b = pool.tile([128, w], F32, tag="nt_b", name="nt_b")
        nc.vector.tensor_mul(b, a, xh)
        y2 = out if it == 1 else pool.tile([128, w], F32, tag="nt_y", name="nt_y")
        nc.vector.scalar_tensor_tensor(
            out=y2, in0=b, scalar=1.5, in1=y, op0=ALU.subtract, op1=ALU.mult
        )
        y = y2
    return out


def _split512(lo, hi):
    """Split [lo, hi) at multiples of 512 (PSUM bank boundaries)."""
    out = []
    a = lo
    while a < hi:
        b = min((a // 512 + 1) * 512, hi)
        out.append((a, b))
        a = b
    return out


# column layout within the packed norm tiles [128, 52]:
# g0 [0:8) g1 [8:16) g2 [16:24) g3 [24:32) Q [32:40) P [40:44) pair [44:52)
_NCOL = {"g0": (0, 8), "g1": (8, 16), "g2": (16, 24), "g3": (24, 32),
         "gq": (32, 40), "gp": (40, 44), "pr": (44, 52)}

# colsum dram layout: diag 896 | k1 1024 | k2 1024 | k3 1024 | P 1024
_COL_OFF = {"diag": 0, "k1": 896, "k2": 1920, "k3": 2944, "p": 3968}


def _build_sym():
    _patch_act_tables()
    nc = bacc.Bacc(
        "TRN2", target_bir_lowering=False, debug=False, num_devices=NCORES
    )
    g_in = [
        nc.dram_tensor(f"g{j}", [128, NB, D], F32, kind="ExternalInput").ap()
        for j in range(4)
    ]
    gq_in = nc.dram_tensor("gq", [128, NB, D], F32, kind="ExternalInput").ap()
    gp_in = nc.dram_tensor("gp", [128, 4, D], F32, kind="ExternalInput").ap()
    pr_in = nc.dram_tensor("pr", [128, NB, D], F32, kind="ExternalInput").ap()
    den_out = nc.dram_tensor("den", [128, 36], F32, kind="ExternalOutput").ap()
    col_out = nc.dram_tensor("col", [1, 4992], F32, kind="ExternalOutput").ap()
    s1_out = nc.dram_tensor("s1p", [1, R], F32, kind="ExternalOutput").ap()
    s2_out = nc.dram_tensor("s2p", [1, R], F32, kind="ExternalOutput").ap()

    with tile.TileContext(nc) as tc, ExitStack() as ctx:
        pers = ctx.enter_context(tc.tile_pool(name="pers", bufs=1))
        spool = ctx.enter_context(tc.tile_pool(name="spool", bufs=2))
        epool = ctx.enter_context(tc.tile_pool(name="epool", bufs=6))
        depool = ctx.enter_context(tc.tile_pool(name="depool", bufs=8))
        pmm = ctx.enter_context(tc.tile_pool(name="pmm", bufs=2, space="PSUM"))
        pcol = ctx.enter_context(tc.tile_pool(name="pcol", bufs=1, space="PSUM"))
        ptx = ctx.enter_context(tc.tile_pool(name="ptx", bufs=2, space="PSUM"))

        ident = pers.tile([128, 128], BF16)
        ones_bf = pers.tile([128, 1], BF16)
        den_sb = pers.tile([128, 36], F32)
        n2 = pers.tile([128, 52], F32)
        n2c = pers.tile([128, 52], F32)
        lns = pers.tile([128, 52], F32)
        inv = pers.tile([128, 52], F32)
        col_sb = pers.tile([1, 4992], F32)
        s_sb = pers.tile([1, 2 * R], F32)

        rm = {}
        for nm, ap_in, nt in (
            ("g0", g_in[0], NB), ("g1", g_in[1], NB), ("g2", g_in[2], NB),
            ("g3", g_in[3], NB), ("gq", gq_in, NB), ("gp", gp_in, 4),
            ("pr", pr_in, NB),
        ):
            rm[nm] = pers.tile([128, nt, D], F32, name=f"rm_{nm}")
        zn = {nm: pers.tile([128, nt, D], BF16, name=f"zn_{nm}")
              for nm, nt in (("g0", NB), ("g1", NB), ("g2", NB), ("g3", NB),
                             ("gq", NB), ("gp", 4))}
        znT = {nm: pers.tile([128, nt * 128], BF16, name=f"znT_{nm}")
               for nm, nt in (("g0", NB), ("g1", NB), ("g2", NB), ("g3", NB),
                              ("gq", NB), ("gp", 4))}

        # ---- input DMAs: g0/g1 race in parallel on separate rings so both
        # are resident before the diag block needs them
        nc.sync.dma_start(out=rm["g0"], in_=g_in[0])
        nc.sync.dma_start(out=rm["g2"], in_=g_in[2])
        nc.sync.dma_start(out=rm["g3"], in_=g_in[3])
        nc.scalar.dma_start(out=rm["g1"], in_=g_in[1])
        nc.scalar.dma_start(out=rm["gq"], in_=gq_in)
        nc.scalar.dma_start(out=rm["gp"], in_=gp_in)
        nc.scalar.dma_start(out=rm["pr"], in_=pr_in)

        make_identity(nc, ident)
        nc.vector.memset(ones_bf, 1.0)
        # warm the (combined ln+exp) table while inputs are still loading
        junk1 = pers.tile([128, 1], F32)
        nc.scalar.activation(out=junk1, in_=ones_bf, func=ACTF.Exp)

        # ---- norm machinery: 1/sqrt(n2) = exp(-0.5 * ln(n2)) on ScalarE
        # (same table set as the main Exp stream -> zero table switches),
        # squares on DVE (critical groups) or GPSIMD, reduces on DVE.
        def norm_sq(nm, tlo, thi, eng):
            a0 = _NCOL[nm][0]
            sq = spool.tile(
                [128, thi - tlo, D], F32, tag="sq", name=f"sq_{nm}{tlo}"
            )
            eng.tensor_mul(sq, rm[nm][:, tlo:thi, :], rm[nm][:, tlo:thi, :])
            nc.vector.tensor_reduce(
                out=n2[:, a0 + tlo: a0 + thi], in_=sq, axis=AX.X, op=ALU.add
            )

        def norm_inv(a, b):
            nc.vector.tensor_scalar_max(n2c[:, a:b], n2[:, a:b], 30.0)
            nc.scalar.activation(out=lns[:, a:b], in_=n2c[:, a:b], func=ACTF.Ln)
            nc.scalar.activation(
                out=inv[:, a:b], in_=lns[:, a:b], func=ACTF.Exp, scale=-0.5
            )

        def zn_mul(nm, tlo=0, thi=None):
            a0 = _NCOL[nm][0]
            if thi is None:
                thi = rm[nm].shape[1]
            nc.vector.tensor_mul(
                zn[nm][:, tlo:thi, :], rm[nm][:, tlo:thi, :],
                _bcast_inner(inv[:, a0 + tlo: a0 + thi], D),
            )

        # two-phase group transpose so the PE work spreads across two
        # emission slots (halves) with the SBUF copy at the end
        _pst = {}

        def tr_half(nm, half):
            nt = zn[nm].shape[1]
            if half == 0:
                _pst[nm] = ptx.tile(
                    [128, nt, 128], BF16, tag="trx", name=f"pst_{nm}"
                )
            pst = _pst[nm]
            h0, h1 = (0, nt // 2) if half == 0 else (nt // 2, nt)
            for t in range(h0, h1):
                nc.tensor.transpose(pst[:, t, :], zn[nm][:, t, :], ident)
            if half == 1:
                nc.vector.tensor_copy(znT[nm], pst)

        # g0 norms, upper half (rows t=4..7) first: the diag block walks
        # strips widest-last so those rows' operands are needed first
        norm_sq("g0", 4, NB, nc.vector)
        norm_inv(4, 8)
        norm_sq("g0", 0, 4, nc.vector)
        norm_inv(0, 4)
        # g1 squares fire on GPSIMD in parallel; reduces are deferred below
        # so they only backfill DVE idle slots (tile scheduling is
        # priority==emission-order among *ready* ops)
        sq1a = spool.tile([128, 4, D], F32, tag="sq1", name="sq1a")
        nc.gpsimd.tensor_mul(sq1a, rm["g1"][:, 0:4, :], rm["g1"][:, 0:4, :])
        sq1b = spool.tile([128, 4, D], F32, tag="sq1", name="sq1b")
        nc.gpsimd.tensor_mul(sq1b, rm["g1"][:, 4:NB, :], rm["g1"][:, 4:NB, :])

        # ---- strip machinery
        def strip(lhsT_src, lt, rhs_src, lo, hi, slot, epool_, etag):
            w = hi - lo
            p = pmm.tile([128, w], F32, tag="ps", name=f"ps_{slot}")
            for a, b in _split512(0, w):
                nc.tensor.matmul(
                    out=p[:, a:b],
                    lhsT=lhsT_src[:, lt * 128:(lt + 1) * 128],
                    rhs=rhs_src[:, lo + a: lo + b],
                    start=True,
                    stop=True,
                )
            e = epool_.tile([128, w], BF16, tag=etag, name=f"e_{slot}")
            nc.scalar.activation(
                out=e, in_=p, func=ACTF.Exp, scale=1.0 / T,
                accum_out=den_sb[:, slot:slot + 1],
            )
            return e

        def colsum_mms(ctile, e, e_off, lo, hi, first_banks, last_banks):
            """ctile[lo:hi) += ones^T @ e[:, e_off + (.-lo)], bank-aware flags.
            first_banks/last_banks: sets of bank indices for which this is
            the first / last accumulating matmul."""
            for a, b in _split512(lo, hi):
                bank = a // 512
                nc.tensor.matmul(
                    out=ctile[:, a:b],
                    lhsT=ones_bf,
                    rhs=e[:, e_off + a - lo: e_off + b - lo],
                    start=bank in first_banks,
                    stop=bank in last_banks,
                )

        # ---- diag block: triangle strips widest-last, zn/transpose/copy
        # emitted per-tile right ahead of each strip; E retained (depool)
        # for the colsum pass that runs between k1 and k2
        pst0 = ptx.tile([128, NB, 128], BF16, tag="trx", name="pst_g0")
        diag_e = {}
        for t in range(NB - 1, -1, -1):
            if t % 2 == 1:
                zn_mul("g0", t - 1, t + 1)
            nc.tensor.transpose(pst0[:, t, :], zn["g0"][:, t, :], ident)
            nc.vector.tensor_copy(
                znT["g0"][:, t * 128:(t + 1) * 128], pst0[:, t, :]
            )
            diag_e[t] = strip(
                znT["g0"], t, znT["g0"], t * 128, 1024, t, depool, "de"
            )
            if t == 2:
                # g1 inv lands mid-diag: reduces backfill DVE gaps, the
                # ln/exp pair slots into the ScalarE act stream here
                for q in range(4):
                    nc.vector.tensor_reduce(
                        out=n2[:, 8 + 2 * q: 10 + 2 * q],
                        in_=(sq1a if q < 2 else sq1b)[:, 2 * (q % 2): 2 * (q % 2) + 2, :],
                        axis=AX.X, op=ALU.add,
                    )
                norm_inv(8, 16)

        zn_mul("g1")
        tr_half("g1", 0)
        tr_half("g1", 1)

        # ---- cyclic blocks k=1..3 + pair block: one-ahead mm emission so
        # PE never in-order-stalls ScalarE
        def block(lhsT_src, rhs_src, slot0, nstrips, ckey, extra=None):
            ctile = pcol.tile([1, 1024], F32, tag="col", name=f"c_{ckey}")
            es = {}
            es[0] = strip(lhsT_src, 0, rhs_src, 0, 1024, slot0, epool, "e")
            for s in range(1, nstrips + 1):
                if s <= nstrips - 1:
                    es[s] = strip(
                        lhsT_src, s, rhs_src, 0, 1024, slot0 + s, epool, "e"
                    )
                if extra is not None and s - 1 in extra:
                    extra[s - 1]()
                colsum_mms(
                    ctile, es[s - 1], 0, 0, 1024,
                    first_banks={0, 1} if s - 1 == 0 else set(),
                    last_banks={0, 1} if s - 1 == nstrips - 1 else set(),
                )
                del es[s - 1]
            nc.vector.tensor_copy(
                col_sb[:, _COL_OFF[ckey]:_COL_OFF[ckey] + 1024], ctile
            )
            nc.gpsimd.dma_start(
                out=col_out[:, _COL_OFF[ckey]:_COL_OFF[ckey] + 1024],
                in_=col_sb[:, _COL_OFF[ckey]:_COL_OFF[ckey] + 1024],
            )

        def late_sq(nm):
            nt = rm[nm].shape[1]
            return lambda: norm_sq(nm, 0, nt, nc.gpsimd)

        def late_reds():
            # halves keep each backfilled DVE op small
            for nm in ("g2", "g3", "gq", "pr"):
                a0 = _NCOL[nm][0]
                for h in (0, 4):
                    sqh = spool.tile(
                        [128, 4, D], F32, tag="sq", name=f"lsq_{nm}{h}"
                    )
                    nc.gpsimd.tensor_mul(
                        sqh, rm[nm][:, h:h + 4, :], rm[nm][:, h:h + 4, :]
                    )
                    nc.vector.tensor_reduce(
                        out=n2[:, a0 + h: a0 + h + 4], in_=sqh,
                        axis=AX.X, op=ALU.add,
                    )
            sqp = spool.tile([128, 4, D], F32, tag="sq", name="lsq_gp")
            nc.gpsimd.tensor_mul(sqp, rm["gp"], rm["gp"])
            nc.vector.tensor_reduce(
                out=n2[:, 40:44], in_=sqp, axis=AX.X, op=ALU.add
            )

        # ---- diag colsums live in two trx-tag banks covering group cols
        # [128,576) and [576,1024); their matmuls interleave into k1's PE
        # slack (slot s handles diag strip t=s-1, ascending so t=0 opens
        # both accumulation banks full-width)
        dcolA = ptx.tile([1, 448], F32, tag="trx", name="dcolA")
        dcolB = ptx.tile([1, 448], F32, tag="trx", name="dcolB")

        def dcol_mms(t):
            # emitted descending t=6..0 (E availability order); start=True
            # zeros the whole bank so the first emitted writer per bank
            # opens it regardless of its column coverage
            e = diag_e[t]
            if t <= 3:
                nc.tensor.matmul(
                    out=dcolA[:, t * 128:448],
                    lhsT=ones_bf,
                    rhs=e[:, 128: 128 + 448 - t * 128],
                    start=t == 3,
                    stop=t == 0,
                )
            lo = max(0, t * 128 - 448)
            e_off = 576 + lo - t * 128
            nc.tensor.matmul(
                out=dcolB[:, lo:448],
                lhsT=ones_bf,
                rhs=e[:, e_off: e_off + 448 - lo],
                start=t == NB - 2,
                stop=t == 0,
            )

        # ---- S1/S2 partials via inv-weighted PE column sums of the raw
        # rows (no zn row-major tensors needed): S[t*128+d] partial
        # = sum_p inv[p,t] * rm[p,t,d]
        def s_half(nm, off, half):
            a0 = _NCOL[nm][0]
            sp = ptx.tile([1, 512], F32, tag="trx", name=f"s_{nm}{half}")
            for tt in range(4):
                t = half * 4 + tt
                nc.tensor.matmul(
                    out=sp[:, tt * 128:(tt + 1) * 128],
                    lhsT=inv[:, a0 + t: a0 + t + 1],
                    rhs=rm[nm][:, t, :],
                    start=tt == 0,
                    stop=tt == 3,
                )
            nc.vector.tensor_copy(
                s_sb[:, off + half * 512: off + (half + 1) * 512], sp
            )

        block(znT["g0"], znT["g1"], 8, NB, "k1", extra={
            1: lambda: (late_reds(), dcol_mms(6))[-1],
            2: lambda: (norm_inv(16, 52), dcol_mms(5))[-1],
            3: lambda: (zn_mul("g2"), zn_mul("gq"), dcol_mms(4))[-1],
            4: lambda: dcol_mms(3),
            5: lambda: (tr_half("g2", 0), dcol_mms(2))[-1],
            6: lambda: (tr_half("g2", 1), dcol_mms(1))[-1],
            7: lambda: dcol_mms(0),
        })
        nc.vector.tensor_copy(col_sb[:, 0:448], dcolA)
        nc.vector.tensor_copy(col_sb[:, 448:896], dcolB)
        nc.gpsimd.dma_start(out=col_out[:, 0:896], in_=col_sb[:, 0:896])

        block(znT["g0"], znT["g2"], 16, NB, "k2", extra={
            1: lambda: zn_mul("g3"),
            3: lambda: tr_half("g3", 0),
            4: lambda: tr_half("g3", 1),
            5: lambda: zn_mul("gp"),
            6: lambda: tr_half("gq", 0),
            7: lambda: tr_half("gq", 1),
        })
        block(znT["g0"], znT["g3"], 24, NB, "k3", extra={
            1: lambda: tr_half("gp", 0),
            2: lambda: tr_half("gp", 1),
            3: lambda: s_half("g0", 0, 0),
            4: lambda: s_half("g0", 0, 1),
            5: lambda: s_half("pr", R, 0),
            6: lambda: s_half("pr", R, 1),
        })
        nc.gpsimd.dma_start(out=den_out[:, 0:32], in_=den_sb[:, 0:32])
        nc.gpsimd.dma_start(out=s1_out, in_=s_sb[:, 0:R])
        nc.gpsimd.dma_start(out=s2_out, in_=s_sb[:, R:2 * R])

        block(znT["gp"], znT["gq"], 32, 4, "p")
        nc.gpsimd.dma_start(out=den_out[:, 32:36], in_=den_sb[:, 32:36])

    nc.compile()
    return nc


def _build_general():
    """Correctness fallback for an arbitrary boolean mask (bf16 0/1 input).
    den correction per row: corr = sum_k mask[j,k] * E[j,k] via DVE
    tensor_tensor_reduce over the exp'd row block."""
    NCHG = 4
    CHG = N // NCHG
    nc = bacc.Bacc(
        "TRN2", target_bir_lowering=False, debug=False, num_devices=NCORES
    )
    nodes_rm = nc.dram_tensor("nodes_rm", [N, D], F32, kind="ExternalInput").ap()
    own_rm = nc.dram_tensor("own_rm", [R, D], F32, kind="ExternalInput").ap()
    pair_rm = nc.dram_tensor("pair_rm", [R, D], F32, kind="ExternalInput").ap()
    mask_bf = nc.dram_tensor("mask_bf", [R, N], BF16, kind="ExternalInput").ap()
    den_out = nc.dram_tensor("den", [128, NB * NCHG], F32, kind="ExternalOutput").ap()
    s1_out = nc.dram_tensor("s1p", [1, R], F32, kind="ExternalOutput").ap()
    s2_out = nc.dram_tensor("s2p", [1, R], F32, kind="ExternalOutput").ap()
    corr_out = nc.dram_tensor("corr", [128, NB], F32, kind="ExternalOutput").ap()

    NT = N // 128

    with tile.TileContext(nc) as tc, ExitStack() as ctx:
        persist = ctx.enter_context(tc.tile_pool(name="persist", bufs=1))
        znT = persist.tile([128, N], BF16)
        own_bf = persist.tile([128, R], BF16)
        inv_all = persist.tile([128, 80], F32)
        inv_ri_T = persist.tile([128, NB], F32)
        den_sb = persist.tile([128, NB, NCHG], F32)
        corr_sb = persist.tile([128, NB], F32)

        with (
            tc.tile_pool(name="pro", bufs=1) as pro,
            tc.tile_pool(name="psum_pro", bufs=1, space="PSUM") as psum_pro,
            tc.tile_pool(name="psum_tr", bufs=2, space="PSUM") as psum_tr,
        ):
            rm_sb = pro.tile([128, NT, D], F32)
            nc.sync.dma_start(
                out=rm_sb, in_=nodes_rm.rearrange("(t p) d -> p t d", p=128)
            )
            own_rm_sb = pro.tile([128, NB, D], F32)
            nc.sync.dma_start(
                out=own_rm_sb, in_=own_rm.rearrange("(t p) d -> p t d", p=128)
            )
            pair_rm_sb = pro.tile([128, NB, D], F32)
            nc.sync.dma_start(
                out=pair_rm_sb, in_=pair_rm.rearrange("(t p) d -> p t d", p=128)
            )

            ident = pro.tile([128, 128], BF16)
            make_identity(nc, ident)
            ones = pro.tile([128, 1], F32)
            nc.vector.memset(ones, 1.0)

            sq = pro.tile([128, NT, D], F32)
            nc.vector.tensor_mul(sq, rm_sb, rm_sb)
            norm2 = pro.tile([128, 80], F32)
            nc.vector.tensor_reduce(
                out=norm2[:, 0:NT], in_=sq, axis=AX.X, op=ALU.add
            )
            sq_own = pro.tile([128, NB, D], F32)
            nc.vector.tensor_mul(sq_own, own_rm_sb, own_rm_sb)
            nc.vector.tensor_reduce(
                out=norm2[:, NT: NT + NB], in_=sq_own, axis=AX.X, op=ALU.add
            )
            sq_pair = pro.tile([128, NB, D], F32)
            nc.vector.tensor_mul(sq_pair, pair_rm_sb, pair_rm_sb)
            nc.vector.tensor_reduce(
                out=norm2[:, NT + NB: NT + 2 * NB],
                in_=sq_pair,
                axis=AX.X,
                op=ALU.add,
            )
            norm2c = pro.tile([128, 80], F32)
            nc.vector.tensor_scalar_max(norm2c, norm2, 30.0)
            _newton_rsqrt(nc, pro, inv_all, norm2c, 80)
            inv_r_pt = inv_all[:, 0:NT]
            inv_ri = inv_all[:, NT: NT + NB]
            inv_rp = inv_all[:, NT + NB: NT + 2 * NB]

            nc.vector.tensor_scalar_mul(inv_ri_T, inv_ri, 1.0 / T)

            zn_rm = pro.tile([128, NT, D], BF16)
            nc.vector.tensor_mul(zn_rm, rm_sb, _bcast_inner(inv_r_pt, D))
            own_rm_bf = pro.tile([128, NB, D], BF16)
            nc.vector.tensor_copy(own_rm_bf, own_rm_sb)

            for g in range(NT // NB):
                pst = psum_tr.tile([128, NB, 128], BF16)
                for t in range(NB):
                    nc.tensor.transpose(
                        pst[:, t, :], zn_rm[:, g * NB + t, :], ident
                    )
                nc.vector.tensor_copy(
                    znT[:, g * NB * 128:(g + 1) * NB * 128], pst
                )
            pst_o = psum_tr.tile([128, NB, 128], BF16)
            for t in range(NB):
                nc.tensor.transpose(pst_o[:, t, :], own_rm_bf[:, t, :], ident)
            nc.vector.tensor_copy(own_bf, pst_o)

            zsc = pro.tile([128, NB, D], F32)
            nc.vector.tensor_mul(zsc, own_rm_sb, _bcast_inner(inv_ri, D))
            zpsc = pro.tile([128, NB, D], F32)
            nc.vector.tensor_mul(zpsc, pair_rm_sb, _bcast_inner(inv_rp, D))
            s1ps = psum_pro.tile([1, R], F32)
            s2ps = psum_pro.tile([1, R], F32)
            zsc_f = zsc.rearrange("p t d -> p (t d)")
            zpsc_f = zpsc.rearrange("p t d -> p (t d)")
            for h in range(R // 512):
                nc.tensor.matmul(
                    out=s1ps[:, h * 512:(h + 1) * 512],
                    lhsT=ones,
                    rhs=zsc_f[:, h * 512:(h + 1) * 512],
                    start=True,
                    stop=True,
                )
                nc.tensor.matmul(
                    out=s2ps[:, h * 512:(h + 1) * 512],
                    lhsT=ones,
                    rhs=zpsc_f[:, h * 512:(h + 1) * 512],
                    start=True,
                    stop=True,
                )
            s1sb = pro.tile([1, R], F32)
            nc.vector.tensor_copy(s1sb, s1ps)
            s2sb = pro.tile([1, R], F32)
            nc.vector.tensor_copy(s2sb, s2ps)
            nc.sync.dma_start(out=s1_out, in_=s1sb)
            nc.sync.dma_start(out=s2_out, in_=s2sb)

        with (
            tc.tile_pool(name="psum_main", bufs=2, space="PSUM") as psum_main,
            tc.tile_pool(name="erow", bufs=2) as epool,
            tc.tile_pool(name="mrow", bufs=2) as mpool,
            tc.tile_pool(name="tjunk", bufs=2) as tjpool,
        ):
            for b in range(NB):
                erow = epool.tile([128, N], BF16)
                mrow = mpool.tile([128, N], BF16)
                nc.sync.dma_start(
                    out=mrow, in_=mask_bf[b * 128:(b + 1) * 128, :]
                )
                for chi in range(NCHG):
                    p = psum_main.tile([128, CHG], F32)
                    for j in range(CHG // 512):
                        k0 = chi * CHG + j * 512
                        nc.tensor.matmul(
                            out=p[:, j * 512:(j + 1) * 512],
                            lhsT=own_bf[:, b * 128:(b + 1) * 128],
                            rhs=znT[:, k0: k0 + 512],
                            start=True,
                            stop=True,
                        )
                    nc.scalar.activation(
                        out=erow[:, chi * CHG:(chi + 1) * CHG],
                        in_=p,
                        func=ACTF.Exp,
                        scale=inv_ri_T[:, b: b + 1],
                        accum_out=den_sb[:, b, chi: chi + 1],
                    )
                tj = tjpool.tile([128, N], BF16)
                nc.vector.tensor_tensor_reduce(
                    out=tj,
                    in0=erow,
                    in1=mrow,
                    scale=1.0,
                    scalar=0.0,
                    op0=ALU.mult,
                    op1=ALU.add,
                    accum_out=corr_sb[:, b: b + 1],
                )
            nc.sync.dma_start(out=den_out, in_=den_sb)
            nc.sync.dma_start(out=corr_out, in_=corr_sb)

    nc.compile()
    return nc


_PROGRAMS = {}


def _program(general: bool):
    if general not in _PROGRAMS:
        _PROGRAMS[general] = _build_general() if general else _build_sym()
    return _PROGRAMS[general]


def kernel(nodes, pair_nodes, nodes_labels, mask):
    global LAST_EXEC_TIME_NS
    nodes = np.ascontiguousarray(np.asarray(nodes), dtype=np.float32)
    pair = np.ascontiguousarray(np.asarray(pair_nodes), dtype=np.float32)
    mask = np.asarray(mask)
    assert nodes.shape == (N, D) and pair.shape == (N, D)

    mask_b = mask.astype(bool, copy=False)
    is_eye = bool(np.count_nonzero(mask_b) == N) and bool(
        mask_b.diagonal().all()
    )

    if not is_eye:
        try:
            mask_bf = mask_b.astype(ml_dtypes.bfloat16)
            return _run_general(nodes, pair, mask_bf)
        except Exception:
            return _host_fallback(nodes, pair, mask_b)
    return _run_eye(nodes, pair)


def _host_fallback(nodes, pair, mask_b):
    """Numpy reference for masks the device fallback cannot handle."""
    def norm_rows(x, eps):
        n = np.linalg.norm(x, axis=1, keepdims=True)
        return x / np.maximum(n, eps)

    n64 = nodes.astype(np.float64)
    p64 = pair.astype(np.float64)
    z = norm_rows(n64, 1e-12)
    zp = norm_rows(p64, 1e-12)
    zn = norm_rows(n64, 1e-8)
    logden = np.empty(N, dtype=np.float64)
    for i in range(0, N, 1024):
        sim = zn[i: i + 1024] @ zn.T
        den = (~mask_b[i: i + 1024] * np.exp(sim / T)).sum(1)
        logden[i: i + 1024] = np.log(den)
    loss = logden.sum() - float(z.sum(0) @ zp.sum(0)) / (N * T)
    return np.float32(loss)


def _run_eye(nodes, pair):
    global LAST_EXEC_TIME_NS
    nc = _program(False)

    # row n = g*1024 + p*8 + t  ->  arr[g][p, t, :]
    arr = nodes.reshape(8, 128, NB, D)
    parr = pair.reshape(8, 128, NB, D)
    in_maps = []
    for c in range(NCORES):
        m = {f"g{j}": np.ascontiguousarray(arr[(c + j) % 8]) for j in range(4)}
        m["gq"] = np.ascontiguousarray(arr[c + 4] if c < 4 else arr[c])
        m["gp"] = np.ascontiguousarray(
            arr[c][:, 0:4] if c < 4 else arr[c - 4][:, 4:8]
        )
        m["pr"] = np.ascontiguousarray(parr[c])
        in_maps.append(m)

    trace = bool(os.environ.get("BASS_TRACE"))
    if trace:
        _install_trace_hook()
    res = run_bass_kernel_spmd(nc, in_maps, list(range(NCORES)), trace=trace)
    LAST_EXEC_TIME_NS = res.exec_time_ns

    den_rows = np.zeros(N, dtype=np.float64)
    q = np.arange(1024)
    perm = (q % 128) * 8 + q // 128        # znT col q -> row offset in group
    S1 = np.zeros(D, dtype=np.float64)
    S2 = np.zeros(D, dtype=np.float64)
    for c in range(NCORES):
        r = res.results[c]
        rs = r["den"].astype(np.float64)                     # [128, 36]
        # diag + k1..k3 rowsums all target own-group rows m*8 + s
        own = rs[:, 0:32].reshape(128, 4, NB).sum(axis=1)    # [m, s]
        den_rows[c * R:(c + 1) * R] += own.reshape(-1)
        # pair-block rowsums: strips s -> t = s (c<4) or s+4 (c>=4)
        pbase = (c if c < 4 else c - 4) * R
        toff = 0 if c < 4 else 4
        pr_ = np.zeros((128, NB))
        pr_[:, toff:toff + 4] = rs[:, 32:36]
        den_rows[pbase:pbase + R] += pr_.reshape(-1)

        col = r["col"].astype(np.float64).reshape(-1)        # [4992]
        den_rows[c * R + perm[128:1024]] += col[0:896]
        for j in (1, 2, 3):
            g = (c + j) % 8
            den_rows[g * R + perm] += col[896 + 1024 * (j - 1): 896 + 1024 * j]
        gq = (c + 4) % 8 if c < 4 else c
        den_rows[gq * R + perm] += col[3968:4992]

        S1 += r["s1p"].astype(np.float64).reshape(NB, D).sum(0)
        S2 += r["s2p"].astype(np.float64).reshape(NB, D).sum(0)

    # exact self-term: device computed exp(sum_d znbf[u,d]^2 / T) with
    # bf16 zn operands and f32 accumulation; reproduce on host
    n64 = nodes.astype(np.float64)
    znb = n64 / np.linalg.norm(n64, axis=1, keepdims=True)
    znb16 = znb.astype(ml_dtypes.bfloat16).astype(np.float64)
    simuu = (znb16 * znb16).sum(1)
    den_rows -= np.exp(simuu / T)

    loss = np.log(den_rows).sum() - float(S1 @ S2) / (N * T)
    return np.float32(loss)


def _run_general(nodes, pair, mask_bf):
    global LAST_EXEC_TIME_NS
    nc = _program(True)

    in_maps = []
    for c in range(NCORES):
        sl = slice(c * R, (c + 1) * R)
        in_maps.append({
            "nodes_rm": nodes,
            "own_rm": np.ascontiguousarray(nodes[sl]),
            "pair_rm": np.ascontiguousarray(pair[sl]),
            "mask_bf": np.ascontiguousarray(mask_bf[sl]),
        })

    trace = bool(os.environ.get("BASS_TRACE"))
    if trace:
        _install_trace_hook()
    res = run_bass_kernel_spmd(nc, in_maps, list(range(NCORES)), trace=trace)
    LAST_EXEC_TIME_NS = res.exec_time_ns

    den_rows = np.empty(N, dtype=np.float64)
    S1 = np.zeros(D, dtype=np.float64)
    S2 = np.zeros(D, dtype=np.float64)
    for c in range(NCORES):
        r = res.results[c]
        den_pb = r["den"].astype(np.float64).reshape(128, NB, 4).sum(-1)
        den_pb -= r["corr"].astype(np.float64)
        # row j = c*1024 + b*128 + p  ->  den_pb[p, b]
        den_rows[c * R:(c + 1) * R] = den_pb.T.reshape(R)
        S1 += r["s1p"].astype(np.float64).reshape(NB, D).sum(0)
        S2 += r["s2p"].astype(np.float64).reshape(NB, D).sum(0)

    loss = np.log(den_rows).sum() - float(S1 @ S2) / (N * T)
    return np.float32(loss)
